# revision 20
# baseline (speedup 1.0000x reference)
"""Trainium2 Bass kernel for nn_Network_56427280335153 (perceiver-style dense transformer).

Sharding: data-parallel over batch B=16 across 8 cores (2 batches/core), no collectives.
Layout: token-major fp32 residual stream; feature-major bf16 operands for matmuls
(out = lhsT.T @ rhs with weights as stationary lhsT); PE transposes between layouts.
"""
import os
import sys
import math
from contextlib import ExitStack

for _p in ("/opt/trn_rl_repo", "/root/.axon_site/_ro/trn_rl_repo"):
    if os.path.isdir(_p) and _p not in sys.path:
        sys.path.insert(0, _p)

import numpy as np
import ml_dtypes

import concourse.bass as bass
import concourse.tile as tile
from concourse import bacc, mybir
from concourse import bass_utils
from concourse.masks import make_identity

F32 = mybir.dt.float32
BF16 = mybir.dt.bfloat16
AF = mybir.ActivationFunctionType
ALU = mybir.AluOpType
AX = mybir.AxisListType

NCORES = 8
B = 2              # batches per core
LAT = 512          # latents per batch
D = 512            # model dim
HEADS = 8
T = B * LAT        # stacked latent tokens per core
CT = 2048          # ctx tokens per batch
QT = 4096          # queries per batch
DEPTH = 4
FF = 4096          # w1 out
FH = 2048          # geglu hidden
KT = D // 128      # 4 feature tiles
EPS = 1e-5
TWO_PI = 2.0 * math.pi
MAGIC = float(1.5 * 2 ** 23)  # fp32 round-to-nearest-integer trick

bf16 = ml_dtypes.bfloat16


class Ker:
    """Builds the per-core Bass program."""

    def __init__(self, dbg=False):
        self.dbg = dbg
        nc = bacc.Bacc("TRN2", target_bir_lowering=False, debug=False)
        self.nc = nc
        self.di = {}

        def inp(name, shape, dt):
            self.di[name] = nc.dram_tensor(name, shape, dt, kind="ExternalInput")

        inp("pts_ctx", (B, CT, 3), F32)
        inp("vals_ctx", (B, CT), F32)
        inp("pts_q", (B, QT, 3), F32)
        inp("alpha", (B,), F32)
        inp("freqs", (256,), F32)
        inp("basis", (3, 24), F32)
        inp("w_pe", (128, D), BF16)
        inp("ctx_bias", (D,), F32)
        inp("qe_bias", (D,), F32)
        inp("lat", (LAT, D), F32)
        for pre in ("ca", "sa"):
            inp(pre + "_wq", (DEPTH, D, D), BF16)
            inp(pre + "_wkv", (DEPTH, D, 2 * D), BF16)
            inp(pre + "_wo", (DEPTH, D, D), BF16)
            inp(pre + "_bo", (DEPTH, D), F32)
        for pre in ("cf", "sf"):
            inp(pre + "_w1", (DEPTH, D, FF), BF16)
            inp(pre + "_b1", (DEPTH, FF), F32)
            inp(pre + "_w2", (DEPTH, FH, D), BF16)
            inp(pre + "_b2", (DEPTH, D), F32)
        inp("w_film", (D, 18 * D), BF16)
        inp("ln_static", (D, 28), F32)
        inp("dec_wq", (D, D), BF16)
        inp("dec_wkv", (D, 2 * D), BF16)
        inp("dec_wo", (D, D), BF16)
        inp("dec_bo", (D,), F32)
        inp("out_w", (D,), BF16)
        inp("out_b", (1,), F32)

        self.out_dram = nc.dram_tensor("out", (B, QT), F32, kind="ExternalOutput")
        self.dbg_drams = {}
        if dbg:
            for nm, shape, dt in [
                ("dbg_embc", (128, B * CT), BF16),
                ("dbg_film", (128, 72, B), F32),
                ("dbg_label", (128, 4, B), BF16),
                ("dbg_x0", (128, 8, D), F32), ("dbg_x1", (128, 8, D), F32),
                ("dbg_x2", (128, 8, D), F32), ("dbg_x3", (128, 8, D), F32),
                ("dbg_x4", (128, 8, D), F32), ("dbg_xf", (128, 8, D), F32),
                ("dbg_xnT", (KT, 128, T), BF16), ("dbg_cnT", (KT, 128, T), BF16),
                ("dbg_qT", (KT, 128, T), BF16), ("dbg_kT", (KT, 128, T), BF16),
                ("dbg_vT", (KT, 128, T), BF16), ("dbg_yT", (KT, 128, T), BF16),
            ]:
                self.dbg_drams[nm] = nc.dram_tensor(nm, shape, dt, kind="ExternalOutput")

        self.ev_ctr = 0
        with ExitStack() as ctx:
            self.ctx = ctx
            self.tc = ctx.enter_context(tile.TileContext(nc))
            self.build()
        nc.finalize()

    # ------------------------------------------------------------------
    def evict(self, out_ap, in_ap):
        self.ev_ctr += 1
        if self.ev_ctr % 2 == 0:
            self.nc.scalar.copy(out_ap, in_ap)
        else:
            self.nc.vector.tensor_copy(out_ap, in_ap)

    def tap(self, name, ap):
        if self.dbg and name in self.dbg_drams:
            self.nc.sync.dma_start(self.dbg_drams[name].ap(), ap)

    def tap_tiles(self, name, tiles):
        if self.dbg and name in self.dbg_drams:
            for i, t_ in enumerate(tiles):
                self.nc.sync.dma_start(self.dbg_drams[name].ap()[i], t_[:])

    # ------------------------------------------------------------------
    def build(self):
        nc, tc, ctx, di = self.nc, self.tc, self.ctx, self.di

        # ---- pools (sized to stay under SBUF budget; see design notes) ----
        self.consts = ctx.enter_context(tc.tile_pool(name="consts", bufs=1))
        self.wp512 = ctx.enter_context(tc.tile_pool(name="wp512", bufs=18))
        self.wp1024 = ctx.enter_context(tc.tile_pool(name="wp1024", bufs=5))
        self.actT = ctx.enter_context(tc.tile_pool(name="actT", bufs=24))
        self.ep = ctx.enter_context(tc.tile_pool(name="ep", bufs=6))
        self.xhp = ctx.enter_context(tc.tile_pool(name="xhp", bufs=8))
        self.svp = ctx.enter_context(tc.tile_pool(name="svp", bufs=16))
        self.sv6p = ctx.enter_context(tc.tile_pool(name="sv6p", bufs=4))
        self.sv2p = ctx.enter_context(tc.tile_pool(name="sv2p", bufs=4))
        self.colp = ctx.enter_context(tc.tile_pool(name="colp", bufs=4))
        self.ps = ctx.enter_context(tc.tile_pool(name="ps", bufs=4, space="PSUM"))
        self.pst = ctx.enter_context(tc.tile_pool(name="pst", bufs=3, space="PSUM"))
        self.psf = ctx.enter_context(tc.tile_pool(name="psf", bufs=1, space="PSUM"))

        consts, svp, colp = self.consts, self.svp, self.colp

        # ---- constants ----
        self.ident = consts.tile([128, 128], BF16, tag="ident")
        make_identity(nc, self.ident[:])
        self.eps_t = consts.tile([128, 1], F32, tag="eps")
        nc.vector.memset(self.eps_t[:], EPS)
        self.negpi = consts.tile([128, 1], F32, tag="negpi")
        nc.vector.memset(self.negpi[:], -math.pi)
        self.w_pe_sb = consts.tile([128, D], BF16, tag="w_pe")
        nc.sync.dma_start(self.w_pe_sb[:], di["w_pe"].ap())
        self.basis_sb = consts.tile([3, 24], F32, tag="basis")
        nc.sync.dma_start(self.basis_sb[:], di["basis"].ap())

        self.lns = []
        for kt in range(KT):
            t_ = consts.tile([128, 28], F32, tag=f"lns{kt}")
            nc.sync.dma_start(t_[:], di["ln_static"].ap()[kt * 128:(kt + 1) * 128, :])
            self.lns.append(t_)

        def bias_bcast(name):
            t_ = consts.tile([128, D], F32, tag=f"bb_{name}")
            src = di[name].ap()
            ap = bass.AP(tensor=src.tensor, offset=src.offset,
                         ap=[[0, 128]] + list(src.ap))
            nc.sync.dma_start(t_[:], ap)
            return t_

        self.ctx_bias_bc = bias_bcast("ctx_bias")
        self.qe_bias_bc = bias_bcast("qe_bias")

        # residual stream x: [128, 8 token-tiles, 512] fp32
        self.x = consts.tile([128, 8, D], F32, tag="x")
        lat_ap = di["lat"].ap()
        for tt in range(8):
            r = (tt % 4) * 128
            nc.sync.dma_start(self.x[:, tt, :], lat_ap[r:r + 128, :])

        # ---- label + film vectors ----
        freq_sb = consts.tile([128, 2], F32, tag="freqs")
        nc.sync.dma_start(freq_sb[:], di["freqs"].ap().rearrange("(c p) -> p c", p=128))
        al_src = di["alpha"].ap()
        alpha_bc = consts.tile([128, B], F32, tag="alpha")
        nc.sync.dma_start(
            alpha_bc[:],
            bass.AP(tensor=al_src.tensor, offset=al_src.offset,
                    ap=[[0, 128]] + list(al_src.ap)))

        labelT = consts.tile([128, 4, B], BF16, tag="labelT")
        for b in range(B):
            t_ = colp.tile([128, 2], F32, tag="lbl_t")
            nc.vector.tensor_scalar_mul(t_[:], freq_sb[:], alpha_bc[:, b:b + 1])
            u_ = colp.tile([128, 2], F32, tag="lbl_u")
            nc.vector.tensor_scalar_add(u_[:], t_[:], 0.25)
            k_ = colp.tile([128, 2], F32, tag="lbl_k")
            nc.vector.tensor_scalar(out=k_[:], in0=t_[:], scalar1=MAGIC, scalar2=MAGIC,
                                    op0=ALU.add, op1=ALU.subtract)
            nc.vector.tensor_sub(t_[:], t_[:], k_[:])
            nc.vector.tensor_scalar(out=k_[:], in0=u_[:], scalar1=MAGIC, scalar2=MAGIC,
                                    op0=ALU.add, op1=ALU.subtract)
            nc.vector.tensor_sub(u_[:], u_[:], k_[:])
            nc.scalar.activation(labelT[:, 0:2, b], u_[:], AF.Sin, scale=TWO_PI)
            nc.scalar.activation(labelT[:, 2:4, b], t_[:], AF.Sin, scale=TWO_PI)

        # film_vecs[p, 72, b] fp32; vec idx = matrix_idx*4 + ft
        self.film = consts.tile([128, 72, B], F32, tag="film")
        wf = di["w_film"].ap()
        for mt in range(72):
            pf = self.psf.tile([128, 512], F32, tag="psf")
            for kt in range(KT):
                wt = self.wp512.tile([128, 128], BF16, tag="wfilm")
                nc.sync.dma_start(wt[:], wf[kt * 128:(kt + 1) * 128,
                                             mt * 128:(mt + 1) * 128])
                nc.tensor.matmul(pf[:, 0:B], wt[:], labelT[:, kt, :],
                                 start=(kt == 0), stop=(kt == KT - 1))
            self.evict(self.film[:, mt, :], pf[:, 0:B])

        # ---- context point-embed table ----
        self.embT_ctx = consts.tile([128, B * CT], BF16, tag="embc")
        self.build_embT(self.embT_ctx, di["pts_ctx"], B * CT, CT, di["vals_ctx"])

        self.tap("dbg_embc", self.embT_ctx[:])
        self.tap("dbg_film", self.film[:])
        self.tap("dbg_label", labelT[:])
        self.tap("dbg_x0", self.x[:])

        # ---- layers ----
        for l in range(DEPTH):
            self.layer(l)

        self.tap("dbg_xf", self.x[:])

        # ---- decode ----
        self.decode()

    # ------------------------------------------------------------------
    def build_embT(self, embT, pts_dram, n_tok, per_b, vals_dram=None):
        """Fill embT [128, n_tok] bf16: rows 0-23 -sin, 32-55 -cos, 64-66 pts, 96 vals/0."""
        nc = self.nc
        with self.tc.tile_pool(name="pep", bufs=2) as pep, \
             self.tc.tile_pool(name="fop", bufs=2) as fop:
            self._embT_body(embT, pts_dram, n_tok, per_b, vals_dram, pep, fop)

    def _embT_body(self, embT, pts_dram, n_tok, per_b, vals_dram, pep, fop):
        nc = self.nc
        self.pep, self.fop = pep, fop
        for ch in range(n_tok // 512):
            b = (ch * 512) // per_b
            t0 = (ch * 512) % per_b
            ptsT = self.pep.tile([3, 512], F32, tag="ptsT")
            for d3 in range(3):
                nc.sync.dma_start(ptsT[d3:d3 + 1, :],
                                  pts_dram.ap()[b, t0:t0 + 512, d3:d3 + 1])
            pp = self.ps.tile([128, 512], F32, tag="ps")
            nc.tensor.matmul(pp[0:24, :], self.basis_sb[:], ptsT[:],
                             start=True, stop=True)
            proj = self.pep.tile([24, 512], F32, tag="proj")
            self.evict(proj[:], pp[0:24, :])
            fc = self.pep.tile([24, 512], F32, tag="frac_c")
            nc.vector.tensor_scalar_add(fc[:], proj[:], 0.25)
            k_ = self.pep.tile([24, 512], F32, tag="kr")
            nc.vector.tensor_scalar(out=k_[:], in0=proj[:], scalar1=MAGIC, scalar2=MAGIC,
                                    op0=ALU.add, op1=ALU.subtract)
            nc.vector.tensor_sub(proj[:], proj[:], k_[:])
            nc.vector.tensor_scalar(out=k_[:], in0=fc[:], scalar1=MAGIC, scalar2=MAGIC,
                                    op0=ALU.add, op1=ALU.subtract)
            nc.vector.tensor_sub(fc[:], fc[:], k_[:])
            sl = slice(ch * 512, ch * 512 + 512)
            nc.scalar.activation(embT[0:24, sl], proj[:], AF.Sin, scale=TWO_PI)
            nc.scalar.activation(embT[32:56, sl], fc[:], AF.Sin, scale=TWO_PI)
            nc.vector.tensor_copy(embT[64:67, sl], ptsT[:])
            if vals_dram is not None:
                vch = self.fop.tile([1, 512], F32, tag="vch")
                nc.sync.dma_start(vch[:], vals_dram.ap()[b, t0:t0 + 512])
                nc.vector.tensor_copy(embT[96:97, sl], vch[:])
        if vals_dram is None:
            nc.vector.memset(embT[96:97, :], 0.0)

    # ------------------------------------------------------------------
    def ln_to_T(self, src_fn, n_tt, S_fn, B_fn):
        """LN token-major tiles -> feature-major bf16 tiles with per-feature affine."""
        nc = self.nc
        xh = []
        for tt in range(n_tt):
            xt = src_fn(tt)
            stats = self.sv6p.tile([128, 6], F32, tag="stats")
            nc.vector.bn_stats(stats[:], xt)
            mv = self.sv2p.tile([128, 2], F32, tag="mv")
            nc.vector.bn_aggr(mv[:], stats[:])
            std = self.svp.tile([128, 1], F32, tag="std")
            nc.scalar.activation(std[:], mv[:, 1:2], AF.Sqrt, bias=self.eps_t[:, 0:1])
            rstd = self.svp.tile([128, 1], F32, tag="rstd")
            nc.vector.reciprocal(rstd[:], std[:])
            nmr = self.svp.tile([128, 1], F32, tag="nmr")
            nc.vector.tensor_scalar(out=nmr[:], in0=mv[:, 0:1], scalar1=rstd[:, 0:1],
                                    scalar2=-1.0, op0=ALU.mult, op1=ALU.mult)
            xh_t = self.xhp.tile([128, 512], BF16, tag="xh")
            nc.scalar.activation(xh_t[:], xt, AF.Identity, bias=nmr[:, 0:1],
                                 scale=rstd[:, 0:1])
            xh.append(xh_t)
        outs = []
        for ft in range(KT):
            o = self.actT.tile([128, n_tt * 128], BF16, tag="actT")
            for q in range(n_tt // 4):
                pq = self.pst.tile([128, 4, 128], BF16, tag="pst")
                for j in range(4):
                    nc.tensor.transpose(pq[:, j, :],
                                        xh[q * 4 + j][:, ft * 128:(ft + 1) * 128],
                                        self.ident[:])
                nc.scalar.activation(o[:, q * 512:(q + 1) * 512], pq[:],
                                     AF.Identity, bias=B_fn(ft, q), scale=S_fn(ft, q))
            outs.append(o)
        return outs

    # ------------------------------------------------------------------
    def mm_to_T(self, w_tiles, rhsT, n_mt, n_cols, bias_fn=None):
        nc = self.nc
        outs = []
        for mt in range(n_mt):
            o = self.actT.tile([128, n_cols], BF16, tag="actT")
            for h in range(n_cols // 512):
                pp = self.ps.tile([128, 512], F32, tag="ps")
                nk = len(w_tiles)
                for kt in range(nk):
                    nc.tensor.matmul(pp[:], w_tiles[kt][:, mt * 128:(mt + 1) * 128],
                                     rhsT[kt][:, h * 512:(h + 1) * 512],
                                     start=(kt == 0), stop=(kt == nk - 1))
                if bias_fn is None:
                    self.evict(o[:, h * 512:(h + 1) * 512], pp[:])
                else:
                    nc.scalar.activation(o[:, h * 512:(h + 1) * 512], pp[:],
                                         AF.Identity, bias=bias_fn(mt))
            outs.append(o)
        return outs

    def load_w(self, dram_ap, n_kt, width, tag, pool):
        tiles = []
        for kt in range(n_kt):
            w = pool.tile([128, width], BF16, tag=tag)
            self.nc.sync.dma_start(w[:], dram_ap[kt * 128:(kt + 1) * 128, :])
            tiles.append(w)
        return tiles

    def load_cols(self, dram_ap, n_cols, dt=F32):
        t_ = self.colp.tile([128, n_cols], dt, tag=f"cols{n_cols}")
        self.nc.sync.dma_start(t_[:], dram_ap.rearrange("(c p) -> p c", p=128))
        return t_

    def transpose_V(self, vT, n_tt, vpool):
        nc = self.nc
        V = vpool.tile([128, n_tt, D], BF16, tag="V")
        for tt in range(n_tt):
            pq = self.pst.tile([128, 4, 128], BF16, tag="pst")
            for ft in range(KT):
                nc.tensor.transpose(pq[:, ft, :], vT[ft][:, tt * 128:(tt + 1) * 128],
                                    self.ident[:])
            self.evict(V[:, tt, :], pq[:])
        return V

    def add_residual(self, yT, n_tt):
        nc = self.nc
        for tt in range(n_tt):
            pq = self.pst.tile([128, 4, 128], BF16, tag="pst")
            for ft in range(KT):
                nc.tensor.transpose(pq[:, ft, :], yT[ft][:, tt * 128:(tt + 1) * 128],
                                    self.ident[:])
            nc.vector.tensor_tensor(out=self.x[:, tt, :], in0=self.x[:, tt, :],
                                    in1=pq[:], op=ALU.add)

    def softmax_rows(self, pp):
        nc = self.nc
        nmax = self.svp.tile([128, 1], F32, tag="nmax")
        nc.vector.reduce_max(nmax[:], pp[:], axis=AX.X, negate=True)
        sums = self.svp.tile([128, 1], F32, tag="sums")
        E = self.ep.tile([128, 512], BF16, tag="E")
        nc.scalar.activation(E[:], pp[:], AF.Exp, bias=nmax[:, 0:1], accum_out=sums[:])
        rec = self.svp.tile([128, 1], F32, tag="rec")
        nc.vector.reciprocal(rec[:], sums[:])
        A = self.ep.tile([128, 512], BF16, tag="E")
        nc.gpsimd.tensor_scalar_mul(A[:], E[:], rec[:, 0:1])
        return A

    # ------------------------------------------------------------------
    def attention(self, qT, kT, vT, wo_tiles, bo_c):
        nc, tc = self.nc, self.tc
        with tc.tile_pool(name="vpool", bufs=1) as vpool, \
             tc.tile_pool(name="atp", bufs=8) as atp:
            V = self.transpose_V(vT, 8, vpool)
            OT = [self.actT.tile([128, T], BF16, tag="actT", name=f"OT{i}")
                  for i in range(KT)]
            for b in range(B):
                ps_o = None
                for h in range(HEADS):
                    ft, sub = h // 2, h % 2
                    po = sub * 64
                    Es = []
                    for qt in range(4):
                        pp = self.ps.tile([128, 512], F32, tag="ps")
                        nc.tensor.matmul(
                            pp[:],
                            qT[ft][po:po + 64,
                                   b * 512 + qt * 128: b * 512 + (qt + 1) * 128],
                            kT[ft][po:po + 64, b * 512:(b + 1) * 512],
                            start=True, stop=True)
                        Es.append(self.softmax_rows(pp))
                    ATt = []
                    for kt in range(4):
                        pq = self.pst.tile([128, 4, 128], BF16, tag="pst")
                        for qt in range(4):
                            nc.tensor.transpose(pq[:, qt, :],
                                                Es[qt][:, kt * 128:(kt + 1) * 128],
                                                self.ident[:])
                        at = atp.tile([128, 512], BF16, tag="AT")
                        self.evict(at[:], pq[:])
                        ATt.append(at)
                    if sub == 0:
                        ps_o = self.ps.tile([128, 512], F32, tag="ps")
                    for jt in range(4):
                        nc.tensor.matmul(ps_o[po:po + 64, :],
                                         V[:, b * 4 + jt, h * 64:(h + 1) * 64],
                                         ATt[jt][:], start=(jt == 0), stop=(jt == 3))
                    if sub == 1:
                        self.evict(OT[ft][:, b * 512:(b + 1) * 512], ps_o[:])
            return self.mm_to_T(wo_tiles, OT, KT, T,
                                bias_fn=lambda mt: bo_c[:, mt:mt + 1])

    # ------------------------------------------------------------------
    def ffn(self, xnT, l, pre):
        nc, tc, di = self.nc, self.tc, self.di
        with tc.tile_pool(name=f"w4k_{pre}{l}", bufs=4) as wp4096, \
             tc.tile_pool(name=f"ag_{pre}{l}", bufs=16) as agp:
            w1 = self.load_w(di[pre + "_w1"].ap()[l], KT, FF, "w1", wp4096)
            b1c = self.load_cols(di[pre + "_b1"].ap()[l], 32)
            ag = []
            for f in range(16):
                o = agp.tile([128, T], BF16, tag="ag")
                for h in range(2):
                    pa = self.ps.tile([128, 512], F32, tag="ps")
                    pg = self.ps.tile([128, 512], F32, tag="ps")
                    for kt in range(KT):
                        nc.tensor.matmul(pa[:], w1[kt][:, f * 128:(f + 1) * 128],
                                         xnT[kt][:, h * 512:(h + 1) * 512],
                                         start=(kt == 0), stop=(kt == KT - 1))
                    for kt in range(KT):
                        nc.tensor.matmul(pg[:], w1[kt][:, (16 + f) * 128:(17 + f) * 128],
                                         xnT[kt][:, h * 512:(h + 1) * 512],
                                         start=(kt == 0), stop=(kt == KT - 1))
                    a_sb = self.ep.tile([128, 512], BF16, tag="E")
                    nc.scalar.activation(a_sb[:], pa[:], AF.Identity,
                                         bias=b1c[:, f:f + 1])
                    g_sb = self.ep.tile([128, 512], BF16, tag="E")
                    nc.scalar.activation(g_sb[:], pg[:], AF.Gelu,
                                         bias=b1c[:, 16 + f:17 + f])
                    nc.vector.tensor_mul(o[:, h * 512:(h + 1) * 512], a_sb[:], g_sb[:])
                ag.append(o)
            w2 = self.load_w(di[pre + "_w2"].ap()[l], FH // 128, D, "w512", self.wp512)
            b2c = self.load_cols(di[pre + "_b2"].ap()[l], 4)
            yT = self.mm_to_T(w2, ag, KT, T, bias_fn=lambda mt: b2c[:, mt:mt + 1])
            self.add_residual(yT, 8)

    # ------------------------------------------------------------------
    def layer(self, l):
        nc, di = self.nc, self.di
        lns = self.lns
        c0 = l * 6
        x_tile = lambda tt: self.x[:, tt, :]

        # cross-attention
        xnT = self.ln_to_T(x_tile, 8,
                           lambda ft, q: lns[ft][:, c0 + 0:c0 + 1],
                           lambda ft, q: lns[ft][:, c0 + 1:c0 + 2])
        with self.tc.tile_pool(name=f"ctxp{l}", bufs=8) as ctxp:
            ctx_tiles = []
            for tt in range(8):
                b, i = tt // 4, tt % 4
                col = b * CT + l * 512 + i * 128
                pp = self.ps.tile([128, 512], F32, tag="ps")
                nc.tensor.matmul(pp[:], self.embT_ctx[:, col:col + 128],
                                 self.w_pe_sb[:], start=True, stop=True)
                ct = ctxp.tile([128, 512], F32, tag="f32t")
                nc.vector.tensor_tensor(out=ct[:], in0=pp[:], in1=self.ctx_bias_bc[:],
                                        op=ALU.add)
                ctx_tiles.append(ct)
            cnT = self.ln_to_T(lambda tt: ctx_tiles[tt][:], 8,
                               lambda ft, q: lns[ft][:, c0 + 2:c0 + 3],
                               lambda ft, q: lns[ft][:, c0 + 3:c0 + 4])
        wq = self.load_w(di["ca_wq"].ap()[l], KT, D, "w512", self.wp512)
        wkv = self.load_w(di["ca_wkv"].ap()[l], KT, 2 * D, "wkv", self.wp1024)
        wo = self.load_w(di["ca_wo"].ap()[l], KT, D, "w512", self.wp512)
        bo_c = self.load_cols(di["ca_bo"].ap()[l], 4)
        qT = self.mm_to_T(wq, xnT, KT, T)
        kvT = self.mm_to_T(wkv, cnT, 2 * KT, T)
        if l == 0:
            self.tap_tiles("dbg_xnT", xnT)
            self.tap_tiles("dbg_cnT", cnT)
            self.tap_tiles("dbg_qT", qT)
            self.tap_tiles("dbg_kT", kvT[:KT])
            self.tap_tiles("dbg_vT", kvT[KT:])
        yT = self.attention(qT, kvT[:KT], kvT[KT:], wo, bo_c)
        if l == 0:
            self.tap_tiles("dbg_yT", yT)
        self.add_residual(yT, 8)
        if l == 0:
            self.tap("dbg_x1", self.x[:])

        # cross FFN
        fnT = self.ln_to_T(x_tile, 8,
                           lambda ft, q: lns[ft][:, c0 + 4:c0 + 5],
                           lambda ft, q: lns[ft][:, c0 + 5:c0 + 6])
        self.ffn(fnT, l, "cf")
        if l == 0:
            self.tap("dbg_x2", self.x[:])

        # FiLM self-attention (film vec idx base: l*16; q index = batch here)
        film = self.film
        sa_s, sa_b = l * 16 + 0, l * 16 + 4
        snT = self.ln_to_T(x_tile, 8,
                           lambda ft, q: film[:, sa_s + ft, q:q + 1],
                           lambda ft, q: film[:, sa_b + ft, q:q + 1])
        wq = self.load_w(di["sa_wq"].ap()[l], KT, D, "w512", self.wp512)
        wkv = self.load_w(di["sa_wkv"].ap()[l], KT, 2 * D, "wkv", self.wp1024)
        wo = self.load_w(di["sa_wo"].ap()[l], KT, D, "w512", self.wp512)
        bo_c = self.load_cols(di["sa_bo"].ap()[l], 4)
        qT = self.mm_to_T(wq, snT, KT, T)
        kvT = self.mm_to_T(wkv, snT, 2 * KT, T)
        yT = self.attention(qT, kvT[:KT], kvT[KT:], wo, bo_c)
        self.add_residual(yT, 8)
        if l == 0:
            self.tap("dbg_x3", self.x[:])

        # FiLM FFN
        sf_s, sf_b = l * 16 + 8, l * 16 + 12
        snT = self.ln_to_T(x_tile, 8,
                           lambda ft, q: film[:, sf_s + ft, q:q + 1],
                           lambda ft, q: film[:, sf_b + ft, q:q + 1])
        self.ffn(snT, l, "sf")
        if l == 0:
            self.tap("dbg_x4", self.x[:])

    # ------------------------------------------------------------------
    def decode(self):
        nc, tc, di = self.nc, self.tc, self.di
        lns, film = self.lns, self.film
        x_tile = lambda tt: self.x[:, tt, :]

        cnT = self.ln_to_T(x_tile, 8,
                           lambda ft, q: lns[ft][:, 26:27],
                           lambda ft, q: lns[ft][:, 27:28])
        dwkv = self.load_w(di["dec_wkv"].ap(), KT, 2 * D, "wkv", self.wp1024)
        kvT = self.mm_to_T(dwkv, cnT, 2 * KT, T)
        kTd, vTd = kvT[:KT], kvT[KT:]
        dwq = self.load_w(di["dec_wq"].ap(), KT, D, "w512", self.wp512)
        dwo = self.load_w(di["dec_wo"].ap(), KT, D, "w512", self.wp512)
        dbo_c = self.load_cols(di["dec_bo"].ap(), 4)
        outw_c = self.load_cols(di["out_w"].ap(), 4, dt=BF16)
        outb_sb = self.consts.tile([1, 1], F32, tag="outb")
        nc.sync.dma_start(outb_sb[:], di["out_b"].ap())

        out_view = self.out_dram.ap().rearrange("b (c t) -> b c t", t=512)

        with tc.tile_pool(name="vpool_d", bufs=1) as vpool, \
             tc.tile_pool(name="atp_d", bufs=8) as atp, \
             tc.tile_pool(name="qep", bufs=6) as qep, \
             tc.tile_pool(name="embq", bufs=1) as embqp:
            Vd = self.transpose_V(vTd, 8, vpool)
            embT_q = embqp.tile([128, B * QT], BF16, tag="embq")
            self.build_embT(embT_q, di["pts_q"], B * QT, QT)

            for qc in range(16):
                b = qc // 8
                qhat = []
                for i in range(4):
                    col = qc * 512 + i * 128
                    pp = self.ps.tile([128, 512], F32, tag="ps")
                    nc.tensor.matmul(pp[:], embT_q[:, col:col + 128], self.w_pe_sb[:],
                                     start=True, stop=True)
                    qe = qep.tile([128, 512], F32, tag="qe")
                    nc.vector.tensor_tensor(out=qe[:], in0=pp[:],
                                            in1=self.qe_bias_bc[:], op=ALU.add)
                    qhat.append(qe)
                qnT = self.ln_to_T(lambda i: qhat[i][:], 4,
                                   lambda ft, q: lns[ft][:, 24:25],
                                   lambda ft, q: lns[ft][:, 25:26])
                qTt = self.mm_to_T(dwq, qnT, KT, 512)
                Es = []
                for i in range(4):
                    pp = self.ps.tile([128, 512], F32, tag="ps")
                    for kt in range(KT):
                        nc.tensor.matmul(pp[:], qTt[kt][:, i * 128:(i + 1) * 128],
                                         kTd[kt][:, b * 512:(b + 1) * 512],
                                         start=(kt == 0), stop=(kt == KT - 1))
                    Es.append(self.softmax_rows(pp))
                ATt = []
                for jt in range(4):
                    pq = self.pst.tile([128, 4, 128], BF16, tag="pst")
                    for i in range(4):
                        nc.tensor.transpose(pq[:, i, :], Es[i][:, jt * 128:(jt + 1) * 128],
                                            self.ident[:])
                    at = atp.tile([128, 512], BF16, tag="AT")
                    self.evict(at[:], pq[:])
                    ATt.append(at)
                OTt = []
                for ft in range(KT):
                    po = self.ps.tile([128, 512], F32, tag="ps")
                    for jt in range(4):
                        nc.tensor.matmul(po[:],
                                         Vd[:, b * 4 + jt, ft * 128:(ft + 1) * 128],
                                         ATt[jt][:], start=(jt == 0), stop=(jt == 3))
                    ot = self.actT.tile([128, 512], BF16, tag="actT")
                    self.evict(ot[:], po[:])
                    OTt.append(ot)
                yTt = self.mm_to_T(dwo, OTt, KT, 512,
                                   bias_fn=lambda mt: dbo_c[:, mt:mt + 1])
                ohat = []
                for i in range(4):
                    pq = self.pst.tile([128, 4, 128], BF16, tag="pst")
                    for ft in range(KT):
                        nc.tensor.transpose(pq[:, ft, :], yTt[ft][:, i * 128:(i + 1) * 128],
                                            self.ident[:])
                    o_sb = qep.tile([128, 512], F32, tag="qe")
                    self.evict(o_sb[:], pq[:])
                    ohat.append(o_sb)
                onT = self.ln_to_T(lambda i: ohat[i][:], 4,
                                   lambda ft, q: film[:, 64 + ft, b:b + 1],
                                   lambda ft, q: film[:, 68 + ft, b:b + 1])
                pf = self.psf.tile([128, 512], F32, tag="psf")
                for kt in range(KT):
                    nc.tensor.matmul(pf[0:1, :], outw_c[:, kt:kt + 1], onT[kt][:],
                                     start=(kt == 0), stop=(kt == KT - 1))
                fo = qep.tile([1, 512], F32, tag="fo")
                nc.scalar.activation(fo[:], pf[0:1, :], AF.Identity,
                                     bias=outb_sb[:, 0:1])
                nc.sync.dma_start(out_view[b, qc % 8, :], fo[:])


# ---------------------------------------------------------------------------
# host side
# ---------------------------------------------------------------------------

_NC_CACHE = None


def _get_nc():
    global _NC_CACHE
    if _NC_CACHE is None:
        _NC_CACHE = Ker().nc
    return _NC_CACHE


def _prep_params(params):
    P = {k: np.asarray(v, dtype=np.float32) for k, v in params.items() if k != "layers"}
    L = {k: np.asarray(v, dtype=np.float32) for k, v in params["layers"].items()}
    d = {}
    d["freqs"] = P["freqs"]
    e = np.power(2.0, np.arange(8)).astype(np.float32) * np.pi
    basis = np.zeros((3, 24), np.float32)
    basis[0, :8] = e
    basis[1, 8:16] = e
    basis[2, 16:] = e
    d["basis"] = basis / TWO_PI
    pe_w = P["pe_w"]
    w128 = np.zeros((128, D), np.float32)
    w128[0:24] = pe_w[0:24]
    w128[32:56] = pe_w[24:48]
    w128[64:67] = pe_w[48:51]
    w128[96:97] = P["ve_w"]
    d["w_pe"] = w128.astype(bf16)
    d["ctx_bias"] = P["pe_b"] + P["ve_b"]
    d["qe_bias"] = P["pe_b"]
    d["lat"] = P["latent"]
    scale = (D // HEADS) ** -0.5
    for pre in ("ca", "sa"):
        d[pre + "_wq"] = (L[pre + "_wq"] * scale).astype(bf16)
        d[pre + "_wkv"] = L[pre + "_wkv"].astype(bf16)
        d[pre + "_wo"] = L[pre + "_wo"].astype(bf16)
        d[pre + "_bo"] = L[pre + "_bo"]
    for pre in ("cf", "sf"):
        d[pre + "_w1"] = L[pre + "_w1"].astype(bf16)
        d[pre + "_b1"] = L[pre + "_b1"]
        d[pre + "_w2"] = L[pre + "_w2"].astype(bf16)
        d[pre + "_b2"] = L[pre + "_b2"]
    mats = []
    for l in range(DEPTH):
        mats.append(L["sa_g"][l] * L["sa_ln_s"][l][None, :])
        mats.append(L["sa_g"][l] * L["sa_ln_b"][l][None, :] + L["sa_be"][l])
        mats.append(L["sf_g"][l] * L["sf_ln_s"][l][None, :])
        mats.append(L["sf_g"][l] * L["sf_ln_b"][l][None, :] + L["sf_be"][l])
    mats.append(P["out_g"] * P["out_ln_s"][None, :])
    mats.append(P["out_g"] * P["out_ln_b"][None, :] + P["out_be"])
    d["w_film"] = np.concatenate(mats, axis=1).astype(bf16)
    cols = []
    for l in range(DEPTH):
        cols += [L["ca_ln_s"][l], L["ca_ln_b"][l], L["ca_lnc_s"][l], L["ca_lnc_b"][l],
                 L["cf_ln_s"][l], L["cf_ln_b"][l]]
    cols += [P["dec_ln_s"], P["dec_ln_b"], P["dec_lnc_s"], P["dec_lnc_b"]]
    d["ln_static"] = np.stack(cols, axis=1)
    d["dec_wq"] = (P["dec_wq"] * (D ** -0.5)).astype(bf16)
    d["dec_wkv"] = P["dec_wkv"].astype(bf16)
    d["dec_wo"] = P["dec_wo"].astype(bf16)
    d["dec_bo"] = P["dec_bo"]
    d["out_w"] = P["out_w"][:, 0].astype(bf16)
    d["out_b"] = P["out_b"]
    return {k: np.ascontiguousarray(v) for k, v in d.items()}


def kernel(context_points, context_values, queries, alpha, params):
    context_points = np.asarray(context_points, np.float32)
    context_values = np.asarray(context_values, np.float32)
    queries = np.asarray(queries, np.float32)
    alpha = np.asarray(alpha, np.float32)

    nc = _get_nc()
    const = _prep_params(params)

    in_maps = []
    for c in range(NCORES):
        s = slice(c * B, (c + 1) * B)
        m = dict(const)
        m["pts_ctx"] = np.ascontiguousarray(context_points[s])
        m["vals_ctx"] = np.ascontiguousarray(context_values[s, :, 0])
        m["pts_q"] = np.ascontiguousarray(queries[s])
        m["alpha"] = np.ascontiguousarray(alpha[s])
        in_maps.append(m)

    res = bass_utils.run_bass_kernel_spmd(nc, in_maps, core_ids=list(range(NCORES)))
    outs = [res.results[c]["out"].reshape(B, QT, 1) for c in range(NCORES)]
    return np.concatenate(outs, axis=0)


if __name__ == "__main__":
    _get_nc()
    print("built ok, instructions:", len(_NC_CACHE.inst_map))


# revision 25
# speedup vs baseline: 2.1239x; 2.1239x over previous
"""Trainium2 Bass kernel for nn_Network_56427280335153 (perceiver-style dense transformer).

Sharding: data-parallel over batch B=16 across 8 cores (2 batches/core), no collectives.
Layout: token-major fp32 residual stream; feature-major bf16 operands for matmuls
(out = lhsT.T @ rhs with weights as stationary lhsT); PE transposes between layouts.
"""
import os
import sys
import math
from contextlib import ExitStack

for _p in ("/opt/trn_rl_repo", "/root/.axon_site/_ro/trn_rl_repo"):
    if os.path.isdir(_p) and _p not in sys.path:
        sys.path.insert(0, _p)

import numpy as np
import ml_dtypes

import concourse.bass as bass
import concourse.tile as tile
from concourse import bacc, mybir
from concourse import bass_utils
from concourse.masks import make_identity

F32 = mybir.dt.float32
BF16 = mybir.dt.bfloat16
AF = mybir.ActivationFunctionType
ALU = mybir.AluOpType
AX = mybir.AxisListType

NCORES = 8
B = 2              # batches per core
LAT = 512          # latents per batch
D = 512            # model dim
HEADS = 8
T = B * LAT        # stacked latent tokens per core
CT = 2048          # ctx tokens per batch
QT = 4096          # queries per batch
DEPTH = 4
FF = 4096          # w1 out
FH = 2048          # geglu hidden
KT = D // 128      # 4 feature tiles
EPS = 1e-5
TWO_PI = 2.0 * math.pi
MAGIC = float(1.5 * 2 ** 23)  # fp32 round-to-nearest-integer trick

bf16 = ml_dtypes.bfloat16


class Ker:
    """Builds the per-core Bass program."""

    def __init__(self, dbg=False):
        self.dbg = dbg
        nc = bacc.Bacc("TRN2", target_bir_lowering=False, debug=False)
        self.nc = nc
        self.di = {}

        def inp(name, shape, dt):
            self.di[name] = nc.dram_tensor(name, shape, dt, kind="ExternalInput")

        inp("pts_ctx", (B, CT, 3), F32)
        inp("vals_ctx", (B, CT), F32)
        inp("pts_q", (B, QT, 3), F32)
        inp("alpha", (B,), F32)
        inp("freqs", (256,), F32)
        inp("basis", (3, 24), F32)
        inp("w_pe", (128, D), BF16)
        inp("ctx_bias", (D,), F32)
        inp("qe_bias", (D,), F32)
        inp("lat", (LAT, D), F32)
        for pre in ("ca", "sa"):
            inp(pre + "_wq", (DEPTH, D, D), BF16)
            inp(pre + "_wkv", (DEPTH, D, 2 * D), BF16)
            inp(pre + "_wo", (DEPTH, D, D), BF16)
            inp(pre + "_bo", (DEPTH, D), F32)
        for pre in ("cf", "sf"):
            inp(pre + "_w1", (DEPTH, D, FF), BF16)
            inp(pre + "_b1", (DEPTH, FF), F32)
            inp(pre + "_w2", (DEPTH, FH, D), BF16)
            inp(pre + "_b2", (DEPTH, D), F32)
        inp("w_film", (D, 18 * D), BF16)
        inp("ln_static", (D, 28), F32)
        inp("dec_wq", (D, D), BF16)
        inp("dec_wkv", (D, 2 * D), BF16)
        inp("dec_wo", (D, D), BF16)
        inp("dec_bo", (D,), F32)
        inp("out_w", (D,), BF16)
        inp("out_b", (1,), F32)
        inp("zerob", (B * QT,), BF16)

        self.out_dram = nc.dram_tensor("out", (B, QT), F32, kind="ExternalOutput")
        self.dbg_drams = {}
        if dbg:
            for nm, shape, dt in [
                ("dbg_embc", (128, B * CT), BF16),
                ("dbg_film", (128, 72, B), F32),
                ("dbg_label", (128, 4, B), BF16),
                ("dbg_x0", (128, 8, D), F32), ("dbg_x1", (128, 8, D), F32),
                ("dbg_x2", (128, 8, D), F32), ("dbg_x3", (128, 8, D), F32),
                ("dbg_x4", (128, 8, D), F32), ("dbg_xf", (128, 8, D), F32),
                ("dbg_xnT", (KT, 128, T), BF16), ("dbg_cnT", (KT, 128, T), BF16),
                ("dbg_ctx0", (8, 128, D), F32), ("dbg_embc2", (128, B * CT), BF16),
                ("dbg_qT", (KT, 128, T), BF16), ("dbg_kT", (KT, 128, T), BF16),
                ("dbg_vT", (KT, 128, T), BF16), ("dbg_yT", (KT, 128, T), BF16),
            ]:
                self.dbg_drams[nm] = nc.dram_tensor(nm, shape, dt, kind="ExternalOutput")

        self.ev_ctr = 0
        with ExitStack() as ctx:
            self.ctx = ctx
            self.tc = ctx.enter_context(tile.TileContext(nc))
            self.build()
        nc.finalize()

    # ------------------------------------------------------------------
    def evict(self, out_ap, in_ap):
        self.ev_ctr += 1
        if self.ev_ctr % 2 == 0:
            self.nc.scalar.copy(out_ap, in_ap)
        else:
            self.nc.vector.tensor_copy(out_ap, in_ap)

    def tap(self, name, ap):
        if self.dbg and name in self.dbg_drams:
            self.nc.sync.dma_start(self.dbg_drams[name].ap(), ap)

    def tap_tiles(self, name, tiles):
        if self.dbg and name in self.dbg_drams:
            for i, t_ in enumerate(tiles):
                self.nc.sync.dma_start(self.dbg_drams[name].ap()[i], t_[:])

    # ------------------------------------------------------------------
    def build(self):
        nc, tc, ctx, di = self.nc, self.tc, self.ctx, self.di

        # ---- pools (sized to stay under SBUF budget; see design notes) ----
        self.consts = ctx.enter_context(tc.tile_pool(name="consts", bufs=1))
        self.wp512 = ctx.enter_context(tc.tile_pool(name="wp512", bufs=18))
        self.wp1024 = ctx.enter_context(tc.tile_pool(name="wp1024", bufs=5))
        self.actT = ctx.enter_context(tc.tile_pool(name="actT", bufs=24))
        self.ep = ctx.enter_context(tc.tile_pool(name="ep", bufs=6))
        self.xhp = ctx.enter_context(tc.tile_pool(name="xhp", bufs=8))
        self.svp = ctx.enter_context(tc.tile_pool(name="svp", bufs=16))
        self.sv6p = ctx.enter_context(tc.tile_pool(name="sv6p", bufs=4))
        self.sv2p = ctx.enter_context(tc.tile_pool(name="sv2p", bufs=4))
        self.colp = ctx.enter_context(tc.tile_pool(name="colp", bufs=4))
        self.ps = ctx.enter_context(tc.tile_pool(name="ps", bufs=4, space="PSUM"))
        self.pst = ctx.enter_context(tc.tile_pool(name="pst", bufs=3, space="PSUM"))
        self.psf = ctx.enter_context(tc.tile_pool(name="psf", bufs=1, space="PSUM"))

        consts, svp, colp = self.consts, self.svp, self.colp

        # ---- constants ----
        self.ident = consts.tile([128, 128], BF16, tag="ident")
        make_identity(nc, self.ident[:])
        self.eps_t = consts.tile([128, 1], F32, tag="eps")
        nc.vector.memset(self.eps_t[:], EPS)
        self.negpi = consts.tile([128, 1], F32, tag="negpi")
        nc.vector.memset(self.negpi[:], -math.pi)
        self.w_pe_sb = consts.tile([128, D], BF16, tag="w_pe")
        nc.sync.dma_start(self.w_pe_sb[:], di["w_pe"].ap())
        self.basis_sb = consts.tile([3, 24], F32, tag="basis")
        nc.sync.dma_start(self.basis_sb[:], di["basis"].ap())

        self.lns = []
        for kt in range(KT):
            t_ = consts.tile([128, 28], F32, tag=f"lns{kt}")
            nc.sync.dma_start(t_[:], di["ln_static"].ap()[kt * 128:(kt + 1) * 128, :])
            self.lns.append(t_)

        def bias_bcast(name):
            t_ = consts.tile([128, D], F32, tag=f"bb_{name}")
            src = di[name].ap()
            ap = bass.AP(tensor=src.tensor, offset=src.offset,
                         ap=[[0, 128]] + list(src.ap))
            nc.sync.dma_start(t_[:], ap)
            return t_

        self.ctx_bias_bc = bias_bcast("ctx_bias")
        self.qe_bias_bc = bias_bcast("qe_bias")

        # residual stream x: [128, 8 token-tiles, 512] fp32
        self.x = consts.tile([128, 8, D], F32, tag="x")
        lat_ap = di["lat"].ap()
        for tt in range(8):
            r = (tt % 4) * 128
            nc.sync.dma_start(self.x[:, tt, :], lat_ap[r:r + 128, :])

        # ---- label + film vectors ----
        freq_sb = consts.tile([128, 2], F32, tag="freqs")
        nc.sync.dma_start(freq_sb[:], di["freqs"].ap().rearrange("(c p) -> p c", p=128))
        al_src = di["alpha"].ap()
        alpha_bc = consts.tile([128, B], F32, tag="alpha")
        nc.sync.dma_start(
            alpha_bc[:],
            bass.AP(tensor=al_src.tensor, offset=al_src.offset,
                    ap=[[0, 128]] + list(al_src.ap)))

        labelT = consts.tile([128, 4, B], BF16, tag="labelT")
        for b in range(B):
            t_ = colp.tile([128, 2], F32, tag="lbl_t")
            nc.vector.tensor_scalar_mul(t_[:], freq_sb[:], alpha_bc[:, b:b + 1])
            u_ = colp.tile([128, 2], F32, tag="lbl_u")
            nc.vector.tensor_scalar_add(u_[:], t_[:], 0.25)
            k_ = colp.tile([128, 2], F32, tag="lbl_k")
            nc.vector.tensor_scalar(out=k_[:], in0=t_[:], scalar1=MAGIC, scalar2=MAGIC,
                                    op0=ALU.add, op1=ALU.subtract)
            nc.vector.tensor_sub(t_[:], t_[:], k_[:])
            nc.vector.tensor_scalar(out=k_[:], in0=u_[:], scalar1=MAGIC, scalar2=MAGIC,
                                    op0=ALU.add, op1=ALU.subtract)
            nc.vector.tensor_sub(u_[:], u_[:], k_[:])
            nc.scalar.activation(labelT[:, 0:2, b], u_[:], AF.Sin, scale=TWO_PI)
            nc.scalar.activation(labelT[:, 2:4, b], t_[:], AF.Sin, scale=TWO_PI)

        # film_vecs[p, 72, b] fp32; vec idx = matrix_idx*4 + ft
        self.film = consts.tile([128, 72, B], F32, tag="film")
        wf = di["w_film"].ap()
        for mt in range(72):
            pf = self.psf.tile([128, 512], F32, tag="psf")
            for kt in range(KT):
                wt = self.wp512.tile([128, 128], BF16, tag="wfilm")
                nc.sync.dma_start(wt[:], wf[kt * 128:(kt + 1) * 128,
                                             mt * 128:(mt + 1) * 128])
                nc.tensor.matmul(pf[:, 0:B], wt[:], labelT[:, kt, :],
                                 start=(kt == 0), stop=(kt == KT - 1))
            self.evict(self.film[:, mt, :], pf[:, 0:B])

        # ---- context point-embed table ----
        self.embT_ctx = consts.tile([128, B * CT], BF16, tag="embc")
        self.build_embT(self.embT_ctx, di["pts_ctx"], B * CT, CT, di["vals_ctx"])

        self.tap("dbg_embc", self.embT_ctx[:])
        self.tap("dbg_film", self.film[:])
        self.tap("dbg_label", labelT[:])
        self.tap("dbg_x0", self.x[:])

        # ---- layers ----
        for l in range(DEPTH):
            self.layer(l)

        self.tap("dbg_xf", self.x[:])

        # ---- decode ----
        self.decode()

    # ------------------------------------------------------------------
    def build_embT(self, embT, pts_dram, n_tok, per_b, vals_dram=None):
        """Fill embT [128, n_tok] bf16: rows 0-23 -sin, 32-55 -cos, 64-66 pts, 96 vals/0."""
        nc = self.nc
        with self.tc.tile_pool(name="pep", bufs=2) as pep, \
             self.tc.tile_pool(name="fop", bufs=2) as fop:
            self._embT_body(embT, pts_dram, n_tok, per_b, vals_dram, pep, fop)

    def _embT_body(self, embT, pts_dram, n_tok, per_b, vals_dram, pep, fop):
        nc = self.nc
        self.pep, self.fop = pep, fop
        zsrc = self.di["zerob"].ap()
        nc.sync.dma_start(embT[:], bass.AP(tensor=zsrc.tensor, offset=zsrc.offset,
                                           ap=[[0, 128], [1, n_tok]]))
        for ch in range(n_tok // 512):
            b = (ch * 512) // per_b
            t0 = (ch * 512) % per_b
            ptsT = self.pep.tile([3, 512], F32, tag="ptsT")
            for d3 in range(3):
                nc.sync.dma_start(ptsT[d3:d3 + 1, :],
                                  pts_dram.ap()[b, t0:t0 + 512, d3:d3 + 1])
            pp = self.ps.tile([128, 512], F32, tag="ps")
            nc.tensor.matmul(pp[0:24, :], self.basis_sb[:], ptsT[:],
                             start=True, stop=True)
            proj = self.pep.tile([24, 512], F32, tag="proj")
            self.evict(proj[:], pp[0:24, :])
            fc = self.pep.tile([24, 512], F32, tag="frac_c")
            nc.vector.tensor_scalar_add(fc[:], proj[:], 0.25)
            k_ = self.pep.tile([24, 512], F32, tag="kr")
            nc.vector.tensor_scalar(out=k_[:], in0=proj[:], scalar1=MAGIC, scalar2=MAGIC,
                                    op0=ALU.add, op1=ALU.subtract)
            nc.vector.tensor_sub(proj[:], proj[:], k_[:])
            nc.vector.tensor_scalar(out=k_[:], in0=fc[:], scalar1=MAGIC, scalar2=MAGIC,
                                    op0=ALU.add, op1=ALU.subtract)
            nc.vector.tensor_sub(fc[:], fc[:], k_[:])
            sl = slice(ch * 512, ch * 512 + 512)
            nc.scalar.activation(embT[0:24, sl], proj[:], AF.Sin, scale=TWO_PI)
            nc.scalar.activation(embT[32:56, sl], fc[:], AF.Sin, scale=TWO_PI)
            nc.vector.tensor_copy(embT[64:67, sl], ptsT[:])
            if vals_dram is not None:
                vch = self.fop.tile([1, 512], F32, tag="vch")
                nc.sync.dma_start(vch[:], vals_dram.ap()[b, t0:t0 + 512])
                nc.vector.tensor_copy(embT[96:97, sl], vch[:])
        if vals_dram is None:
            nc.vector.memset(embT[96:97, :], 0.0)

    # ------------------------------------------------------------------
    def ln_to_T(self, src_fn, n_tt, S_fn, B_fn):
        """LN token-major tiles -> feature-major bf16 tiles with per-feature affine."""
        nc = self.nc
        xh = []
        for tt in range(n_tt):
            xt = src_fn(tt)
            stats = self.sv6p.tile([128, 6], F32, tag="stats")
            nc.vector.bn_stats(stats[:], xt)
            mv = self.sv2p.tile([128, 2], F32, tag="mv")
            nc.vector.bn_aggr(mv[:], stats[:])
            std = self.svp.tile([128, 1], F32, tag="std")
            nc.scalar.activation(std[:], mv[:, 1:2], AF.Sqrt, bias=self.eps_t[:, 0:1])
            rstd = self.svp.tile([128, 1], F32, tag="rstd")
            nc.vector.reciprocal(rstd[:], std[:])
            nmr = self.svp.tile([128, 1], F32, tag="nmr")
            nc.vector.tensor_scalar(out=nmr[:], in0=mv[:, 0:1], scalar1=rstd[:, 0:1],
                                    scalar2=-1.0, op0=ALU.mult, op1=ALU.mult)
            xh_t = self.xhp.tile([128, 512], BF16, tag="xh")
            nc.scalar.activation(xh_t[:], xt, AF.Identity, bias=nmr[:, 0:1],
                                 scale=rstd[:, 0:1])
            xh.append(xh_t)
        outs = []
        for ft in range(KT):
            o = self.actT.tile([128, n_tt * 128], BF16, tag="actT")
            for q in range(n_tt // 4):
                pq = self.pst.tile([128, 4, 128], BF16, tag="pst")
                for j in range(4):
                    nc.tensor.transpose(pq[:, j, :],
                                        xh[q * 4 + j][:, ft * 128:(ft + 1) * 128],
                                        self.ident[:])
                nc.scalar.activation(o[:, q * 512:(q + 1) * 512], pq[:],
                                     AF.Identity, bias=B_fn(ft, q), scale=S_fn(ft, q))
            outs.append(o)
        return outs

    # ------------------------------------------------------------------
    def mm_to_T(self, w_tiles, rhsT, n_mt, n_cols, bias_fn=None):
        nc = self.nc
        outs = []
        for mt in range(n_mt):
            o = self.actT.tile([128, n_cols], BF16, tag="actT")
            for h in range(n_cols // 512):
                pp = self.ps.tile([128, 512], F32, tag="ps")
                nk = len(w_tiles)
                for kt in range(nk):
                    nc.tensor.matmul(pp[:], w_tiles[kt][:, mt * 128:(mt + 1) * 128],
                                     rhsT[kt][:, h * 512:(h + 1) * 512],
                                     start=(kt == 0), stop=(kt == nk - 1))
                if bias_fn is None:
                    self.evict(o[:, h * 512:(h + 1) * 512], pp[:])
                else:
                    nc.scalar.activation(o[:, h * 512:(h + 1) * 512], pp[:],
                                         AF.Identity, bias=bias_fn(mt))
            outs.append(o)
        return outs

    def load_w(self, dram_ap, n_kt, width, tag, pool):
        tiles = []
        for kt in range(n_kt):
            w = pool.tile([128, width], BF16, tag=tag)
            self.nc.sync.dma_start(w[:], dram_ap[kt * 128:(kt + 1) * 128, :])
            tiles.append(w)
        return tiles

    def load_cols(self, dram_ap, n_cols, dt=F32):
        t_ = self.colp.tile([128, n_cols], dt, tag=f"cols{n_cols}")
        self.nc.sync.dma_start(t_[:], dram_ap.rearrange("(c p) -> p c", p=128))
        return t_

    def transpose_V(self, vT, n_tt, vpool):
        nc = self.nc
        V = vpool.tile([128, n_tt, D], BF16, tag="V")
        for tt in range(n_tt):
            pq = self.pst.tile([128, 4, 128], BF16, tag="pst")
            for ft in range(KT):
                nc.tensor.transpose(pq[:, ft, :], vT[ft][:, tt * 128:(tt + 1) * 128],
                                    self.ident[:])
            self.evict(V[:, tt, :], pq[:])
        return V

    def add_residual(self, yT, n_tt):
        nc = self.nc
        for tt in range(n_tt):
            pq = self.pst.tile([128, 4, 128], BF16, tag="pst")
            for ft in range(KT):
                nc.tensor.transpose(pq[:, ft, :], yT[ft][:, tt * 128:(tt + 1) * 128],
                                    self.ident[:])
            nc.vector.tensor_tensor(out=self.x[:, tt, :], in0=self.x[:, tt, :],
                                    in1=pq[:], op=ALU.add)

    def softmax_rows(self, pp):
        nc = self.nc
        nmax = self.svp.tile([128, 1], F32, tag="nmax")
        nc.vector.reduce_max(nmax[:], pp[:], axis=AX.X, negate=True)
        sums = self.svp.tile([128, 1], F32, tag="sums")
        E = self.ep.tile([128, 512], BF16, tag="E")
        nc.scalar.activation(E[:], pp[:], AF.Exp, bias=nmax[:, 0:1], accum_out=sums[:])
        rec = self.svp.tile([128, 1], F32, tag="rec")
        nc.vector.reciprocal(rec[:], sums[:])
        A = self.ep.tile([128, 512], BF16, tag="E")
        nc.vector.tensor_scalar_mul(A[:], E[:], rec[:, 0:1])
        return A

    # ------------------------------------------------------------------
    def attention(self, qT, kT, vT, wo_tiles, bo_c):
        nc, tc = self.nc, self.tc
        with tc.tile_pool(name="vpool", bufs=1) as vpool, \
             tc.tile_pool(name="atp", bufs=8) as atp:
            V = self.transpose_V(vT, 8, vpool)
            OT = [self.actT.tile([128, T], BF16, tag="actT", name=f"OT{i}")
                  for i in range(KT)]
            for b in range(B):
                ps_o = None
                for h in range(HEADS):
                    ft, sub = h // 2, h % 2
                    po = sub * 64
                    Es = []
                    for qt in range(4):
                        pp = self.ps.tile([128, 512], F32, tag="ps")
                        nc.tensor.matmul(
                            pp[:],
                            qT[ft][po:po + 64,
                                   b * 512 + qt * 128: b * 512 + (qt + 1) * 128],
                            kT[ft][po:po + 64, b * 512:(b + 1) * 512],
                            start=True, stop=True)
                        Es.append(self.softmax_rows(pp))
                    ATt = []
                    for kt in range(4):
                        pq = self.pst.tile([128, 4, 128], BF16, tag="pst")
                        for qt in range(4):
                            nc.tensor.transpose(pq[:, qt, :],
                                                Es[qt][:, kt * 128:(kt + 1) * 128],
                                                self.ident[:])
                        at = atp.tile([128, 512], BF16, tag="AT")
                        self.evict(at[:], pq[:])
                        ATt.append(at)
                    if sub == 0:
                        ps_o = self.ps.tile([128, 512], F32, tag="ps")
                    for jt in range(4):
                        nc.tensor.matmul(ps_o[po:po + 64, :],
                                         V[:, b * 4 + jt, h * 64:(h + 1) * 64],
                                         ATt[jt][:], start=(jt == 0), stop=(jt == 3))
                    if sub == 1:
                        self.evict(OT[ft][:, b * 512:(b + 1) * 512], ps_o[:])
            return self.mm_to_T(wo_tiles, OT, KT, T,
                                bias_fn=lambda mt: bo_c[:, mt:mt + 1])

    # ------------------------------------------------------------------
    def ffn(self, xnT, l, pre):
        nc, tc, di = self.nc, self.tc, self.di
        with tc.tile_pool(name=f"w4k_{pre}{l}", bufs=4) as wp4096, \
             tc.tile_pool(name=f"ag_{pre}{l}", bufs=16) as agp:
            w1 = self.load_w(di[pre + "_w1"].ap()[l], KT, FF, "w1", wp4096)
            b1c = self.load_cols(di[pre + "_b1"].ap()[l], 32)
            ag = []
            for f in range(16):
                o = agp.tile([128, T], BF16, tag="ag")
                for h in range(2):
                    pa = self.ps.tile([128, 512], F32, tag="ps")
                    pg = self.ps.tile([128, 512], F32, tag="ps")
                    for kt in range(KT):
                        nc.tensor.matmul(pa[:], w1[kt][:, f * 128:(f + 1) * 128],
                                         xnT[kt][:, h * 512:(h + 1) * 512],
                                         start=(kt == 0), stop=(kt == KT - 1))
                    for kt in range(KT):
                        nc.tensor.matmul(pg[:], w1[kt][:, (16 + f) * 128:(17 + f) * 128],
                                         xnT[kt][:, h * 512:(h + 1) * 512],
                                         start=(kt == 0), stop=(kt == KT - 1))
                    a_sb = self.ep.tile([128, 512], BF16, tag="E")
                    nc.scalar.activation(a_sb[:], pa[:], AF.Identity,
                                         bias=b1c[:, f:f + 1])
                    g_sb = self.ep.tile([128, 512], BF16, tag="E")
                    nc.scalar.activation(g_sb[:], pg[:], AF.Gelu,
                                         bias=b1c[:, 16 + f:17 + f])
                    nc.vector.tensor_mul(o[:, h * 512:(h + 1) * 512], a_sb[:], g_sb[:])
                ag.append(o)
            w2 = self.load_w(di[pre + "_w2"].ap()[l], FH // 128, D, "w512", self.wp512)
            b2c = self.load_cols(di[pre + "_b2"].ap()[l], 4)
            yT = self.mm_to_T(w2, ag, KT, T, bias_fn=lambda mt: b2c[:, mt:mt + 1])
            self.add_residual(yT, 8)

    # ------------------------------------------------------------------
    def layer(self, l):
        nc, di = self.nc, self.di
        lns = self.lns
        c0 = l * 6
        x_tile = lambda tt: self.x[:, tt, :]

        # cross-attention
        xnT = self.ln_to_T(x_tile, 8,
                           lambda ft, q: lns[ft][:, c0 + 0:c0 + 1],
                           lambda ft, q: lns[ft][:, c0 + 1:c0 + 2])
        with self.tc.tile_pool(name=f"ctxp{l}", bufs=8) as ctxp:
            ctx_tiles = []
            for tt in range(8):
                b, i = tt // 4, tt % 4
                col = b * CT + l * 512 + i * 128
                pp = self.ps.tile([128, 512], F32, tag="ps")
                nc.tensor.matmul(pp[:], self.embT_ctx[:, col:col + 128],
                                 self.w_pe_sb[:], start=True, stop=True)
                ct = ctxp.tile([128, 512], F32, tag="f32t")
                nc.vector.tensor_tensor(out=ct[:], in0=pp[:], in1=self.ctx_bias_bc[:],
                                        op=ALU.add)
                ctx_tiles.append(ct)
            if l == 0:
                self.tap("dbg_embc2", self.embT_ctx[:])
                self.tap_tiles("dbg_ctx0", ctx_tiles)
            cnT = self.ln_to_T(lambda tt: ctx_tiles[tt][:], 8,
                               lambda ft, q: lns[ft][:, c0 + 2:c0 + 3],
                               lambda ft, q: lns[ft][:, c0 + 3:c0 + 4])
        wq = self.load_w(di["ca_wq"].ap()[l], KT, D, "w512", self.wp512)
        wkv = self.load_w(di["ca_wkv"].ap()[l], KT, 2 * D, "wkv", self.wp1024)
        wo = self.load_w(di["ca_wo"].ap()[l], KT, D, "w512", self.wp512)
        bo_c = self.load_cols(di["ca_bo"].ap()[l], 4)
        qT = self.mm_to_T(wq, xnT, KT, T)
        kvT = self.mm_to_T(wkv, cnT, 2 * KT, T)
        if l == 0:
            self.tap_tiles("dbg_xnT", xnT)
            self.tap_tiles("dbg_cnT", cnT)
            self.tap_tiles("dbg_qT", qT)
            self.tap_tiles("dbg_kT", kvT[:KT])
            self.tap_tiles("dbg_vT", kvT[KT:])
        yT = self.attention(qT, kvT[:KT], kvT[KT:], wo, bo_c)
        if l == 0:
            self.tap_tiles("dbg_yT", yT)
        self.add_residual(yT, 8)
        if l == 0:
            self.tap("dbg_x1", self.x[:])

        # cross FFN
        fnT = self.ln_to_T(x_tile, 8,
                           lambda ft, q: lns[ft][:, c0 + 4:c0 + 5],
                           lambda ft, q: lns[ft][:, c0 + 5:c0 + 6])
        self.ffn(fnT, l, "cf")
        if l == 0:
            self.tap("dbg_x2", self.x[:])

        # FiLM self-attention (film vec idx base: l*16; q index = batch here)
        film = self.film
        sa_s, sa_b = l * 16 + 0, l * 16 + 4
        snT = self.ln_to_T(x_tile, 8,
                           lambda ft, q: film[:, sa_s + ft, q:q + 1],
                           lambda ft, q: film[:, sa_b + ft, q:q + 1])
        wq = self.load_w(di["sa_wq"].ap()[l], KT, D, "w512", self.wp512)
        wkv = self.load_w(di["sa_wkv"].ap()[l], KT, 2 * D, "wkv", self.wp1024)
        wo = self.load_w(di["sa_wo"].ap()[l], KT, D, "w512", self.wp512)
        bo_c = self.load_cols(di["sa_bo"].ap()[l], 4)
        qT = self.mm_to_T(wq, snT, KT, T)
        kvT = self.mm_to_T(wkv, snT, 2 * KT, T)
        yT = self.attention(qT, kvT[:KT], kvT[KT:], wo, bo_c)
        self.add_residual(yT, 8)
        if l == 0:
            self.tap("dbg_x3", self.x[:])

        # FiLM FFN
        sf_s, sf_b = l * 16 + 8, l * 16 + 12
        snT = self.ln_to_T(x_tile, 8,
                           lambda ft, q: film[:, sf_s + ft, q:q + 1],
                           lambda ft, q: film[:, sf_b + ft, q:q + 1])
        self.ffn(snT, l, "sf")
        if l == 0:
            self.tap("dbg_x4", self.x[:])

    # ------------------------------------------------------------------
    def decode(self):
        nc, tc, di = self.nc, self.tc, self.di
        lns, film = self.lns, self.film
        x_tile = lambda tt: self.x[:, tt, :]

        cnT = self.ln_to_T(x_tile, 8,
                           lambda ft, q: lns[ft][:, 26:27],
                           lambda ft, q: lns[ft][:, 27:28])
        dwkv = self.load_w(di["dec_wkv"].ap(), KT, 2 * D, "wkv", self.wp1024)
        kvT = self.mm_to_T(dwkv, cnT, 2 * KT, T)
        kTd, vTd = kvT[:KT], kvT[KT:]
        dwq = self.load_w(di["dec_wq"].ap(), KT, D, "w512", self.wp512)
        dwo = self.load_w(di["dec_wo"].ap(), KT, D, "w512", self.wp512)
        dbo_c = self.load_cols(di["dec_bo"].ap(), 4)
        outw_c = self.load_cols(di["out_w"].ap(), 4, dt=BF16)
        outb_sb = self.consts.tile([1, 1], F32, tag="outb")
        nc.sync.dma_start(outb_sb[:], di["out_b"].ap())

        out_view = self.out_dram.ap().rearrange("b (c t) -> b c t", t=512)

        with tc.tile_pool(name="vpool_d", bufs=1) as vpool, \
             tc.tile_pool(name="atp_d", bufs=8) as atp, \
             tc.tile_pool(name="qep", bufs=6) as qep, \
             tc.tile_pool(name="embq", bufs=1) as embqp:
            Vd = self.transpose_V(vTd, 8, vpool)
            embT_q = embqp.tile([128, B * QT], BF16, tag="embq")
            self.build_embT(embT_q, di["pts_q"], B * QT, QT)

            for qc in range(16):
                b = qc // 8
                qhat = []
                for i in range(4):
                    col = qc * 512 + i * 128
                    pp = self.ps.tile([128, 512], F32, tag="ps")
                    nc.tensor.matmul(pp[:], embT_q[:, col:col + 128], self.w_pe_sb[:],
                                     start=True, stop=True)
                    qe = qep.tile([128, 512], F32, tag="qe")
                    nc.vector.tensor_tensor(out=qe[:], in0=pp[:],
                                            in1=self.qe_bias_bc[:], op=ALU.add)
                    qhat.append(qe)
                qnT = self.ln_to_T(lambda i: qhat[i][:], 4,
                                   lambda ft, q: lns[ft][:, 24:25],
                                   lambda ft, q: lns[ft][:, 25:26])
                qTt = self.mm_to_T(dwq, qnT, KT, 512)
                Es = []
                for i in range(4):
                    pp = self.ps.tile([128, 512], F32, tag="ps")
                    for kt in range(KT):
                        nc.tensor.matmul(pp[:], qTt[kt][:, i * 128:(i + 1) * 128],
                                         kTd[kt][:, b * 512:(b + 1) * 512],
                                         start=(kt == 0), stop=(kt == KT - 1))
                    Es.append(self.softmax_rows(pp))
                ATt = []
                for jt in range(4):
                    pq = self.pst.tile([128, 4, 128], BF16, tag="pst")
                    for i in range(4):
                        nc.tensor.transpose(pq[:, i, :], Es[i][:, jt * 128:(jt + 1) * 128],
                                            self.ident[:])
                    at = atp.tile([128, 512], BF16, tag="AT")
                    self.evict(at[:], pq[:])
                    ATt.append(at)
                OTt = []
                for ft in range(KT):
                    po = self.ps.tile([128, 512], F32, tag="ps")
                    for jt in range(4):
                        nc.tensor.matmul(po[:],
                                         Vd[:, b * 4 + jt, ft * 128:(ft + 1) * 128],
                                         ATt[jt][:], start=(jt == 0), stop=(jt == 3))
                    ot = self.actT.tile([128, 512], BF16, tag="actT")
                    self.evict(ot[:], po[:])
                    OTt.append(ot)
                yTt = self.mm_to_T(dwo, OTt, KT, 512,
                                   bias_fn=lambda mt: dbo_c[:, mt:mt + 1])
                ohat = []
                for i in range(4):
                    pq = self.pst.tile([128, 4, 128], BF16, tag="pst")
                    for ft in range(KT):
                        nc.tensor.transpose(pq[:, ft, :], yTt[ft][:, i * 128:(i + 1) * 128],
                                            self.ident[:])
                    o_sb = qep.tile([128, 512], F32, tag="qe")
                    self.evict(o_sb[:], pq[:])
                    ohat.append(o_sb)
                onT = self.ln_to_T(lambda i: ohat[i][:], 4,
                                   lambda ft, q: film[:, 64 + ft, b:b + 1],
                                   lambda ft, q: film[:, 68 + ft, b:b + 1])
                pf = self.psf.tile([128, 512], F32, tag="psf")
                for kt in range(KT):
                    nc.tensor.matmul(pf[0:1, :], outw_c[:, kt:kt + 1], onT[kt][:],
                                     start=(kt == 0), stop=(kt == KT - 1))
                fo = qep.tile([1, 512], F32, tag="fo")
                nc.scalar.activation(fo[:], pf[0:1, :], AF.Identity,
                                     bias=outb_sb[:, 0:1])
                nc.sync.dma_start(out_view[b, qc % 8, :], fo[:])


# ---------------------------------------------------------------------------
# host side
# ---------------------------------------------------------------------------

_NC_CACHE = None


def _get_nc():
    global _NC_CACHE
    if _NC_CACHE is None:
        _NC_CACHE = Ker().nc
    return _NC_CACHE


def _prep_params(params):
    P = {k: np.asarray(v, dtype=np.float32) for k, v in params.items() if k != "layers"}
    L = {k: np.asarray(v, dtype=np.float32) for k, v in params["layers"].items()}
    d = {}
    d["freqs"] = P["freqs"]
    e = np.power(2.0, np.arange(8)).astype(np.float32) * np.pi
    basis = np.zeros((3, 24), np.float32)
    basis[0, :8] = e
    basis[1, 8:16] = e
    basis[2, 16:] = e
    d["basis"] = basis / TWO_PI
    pe_w = P["pe_w"]
    w128 = np.zeros((128, D), np.float32)
    w128[0:24] = pe_w[0:24]
    w128[32:56] = pe_w[24:48]
    w128[64:67] = pe_w[48:51]
    w128[96:97] = P["ve_w"]
    d["w_pe"] = w128.astype(bf16)
    d["ctx_bias"] = P["pe_b"] + P["ve_b"]
    d["qe_bias"] = P["pe_b"]
    d["lat"] = P["latent"]
    scale = (D // HEADS) ** -0.5
    for pre in ("ca", "sa"):
        d[pre + "_wq"] = (L[pre + "_wq"] * scale).astype(bf16)
        d[pre + "_wkv"] = L[pre + "_wkv"].astype(bf16)
        d[pre + "_wo"] = L[pre + "_wo"].astype(bf16)
        d[pre + "_bo"] = L[pre + "_bo"]
    for pre in ("cf", "sf"):
        d[pre + "_w1"] = L[pre + "_w1"].astype(bf16)
        d[pre + "_b1"] = L[pre + "_b1"]
        d[pre + "_w2"] = L[pre + "_w2"].astype(bf16)
        d[pre + "_b2"] = L[pre + "_b2"]
    mats = []
    for l in range(DEPTH):
        mats.append(L["sa_g"][l] * L["sa_ln_s"][l][None, :])
        mats.append(L["sa_g"][l] * L["sa_ln_b"][l][None, :] + L["sa_be"][l])
        mats.append(L["sf_g"][l] * L["sf_ln_s"][l][None, :])
        mats.append(L["sf_g"][l] * L["sf_ln_b"][l][None, :] + L["sf_be"][l])
    mats.append(P["out_g"] * P["out_ln_s"][None, :])
    mats.append(P["out_g"] * P["out_ln_b"][None, :] + P["out_be"])
    d["w_film"] = np.concatenate(mats, axis=1).astype(bf16)
    cols = []
    for l in range(DEPTH):
        cols += [L["ca_ln_s"][l], L["ca_ln_b"][l], L["ca_lnc_s"][l], L["ca_lnc_b"][l],
                 L["cf_ln_s"][l], L["cf_ln_b"][l]]
    cols += [P["dec_ln_s"], P["dec_ln_b"], P["dec_lnc_s"], P["dec_lnc_b"]]
    d["ln_static"] = np.stack(cols, axis=1)
    d["dec_wq"] = (P["dec_wq"] * (D ** -0.5)).astype(bf16)
    d["dec_wkv"] = P["dec_wkv"].astype(bf16)
    d["dec_wo"] = P["dec_wo"].astype(bf16)
    d["dec_bo"] = P["dec_bo"]
    d["out_w"] = P["out_w"][:, 0].astype(bf16)
    d["out_b"] = P["out_b"]
    d["zerob"] = np.zeros((B * QT,), bf16)
    return {k: np.ascontiguousarray(v) for k, v in d.items()}


def kernel(context_points, context_values, queries, alpha, params):
    context_points = np.asarray(context_points, np.float32)
    context_values = np.asarray(context_values, np.float32)
    queries = np.asarray(queries, np.float32)
    alpha = np.asarray(alpha, np.float32)

    nc = _get_nc()
    const = _prep_params(params)

    in_maps = []
    for c in range(NCORES):
        s = slice(c * B, (c + 1) * B)
        m = dict(const)
        m["pts_ctx"] = np.ascontiguousarray(context_points[s])
        m["vals_ctx"] = np.ascontiguousarray(context_values[s, :, 0])
        m["pts_q"] = np.ascontiguousarray(queries[s])
        m["alpha"] = np.ascontiguousarray(alpha[s])
        in_maps.append(m)

    res = bass_utils.run_bass_kernel_spmd(nc, in_maps, core_ids=list(range(NCORES)))
    outs = [res.results[c]["out"].reshape(B, QT, 1) for c in range(NCORES)]
    return np.concatenate(outs, axis=0)


if __name__ == "__main__":
    _get_nc()
    print("built ok, instructions:", len(_NC_CACHE.inst_map))


# revision 26
# speedup vs baseline: 2.2091x; 1.0401x over previous
"""Trainium2 Bass kernel for nn_Network_56427280335153 (perceiver-style dense transformer).

Sharding: data-parallel over batch B=16 across 8 cores (2 batches/core), no collectives.
Layout: token-major fp32 residual stream; feature-major bf16 operands for matmuls
(out = lhsT.T @ rhs with weights as stationary lhsT); PE transposes between layouts.
"""
import os
import sys
import math
from contextlib import ExitStack

for _p in ("/opt/trn_rl_repo", "/root/.axon_site/_ro/trn_rl_repo"):
    if os.path.isdir(_p) and _p not in sys.path:
        sys.path.insert(0, _p)

import numpy as np
import ml_dtypes

import concourse.bass as bass
import concourse.tile as tile
from concourse import bacc, mybir
from concourse import bass_utils
from concourse.masks import make_identity

F32 = mybir.dt.float32
BF16 = mybir.dt.bfloat16
AF = mybir.ActivationFunctionType
ALU = mybir.AluOpType
AX = mybir.AxisListType

NCORES = 8
B = 2              # batches per core
LAT = 512          # latents per batch
D = 512            # model dim
HEADS = 8
T = B * LAT        # stacked latent tokens per core
CT = 2048          # ctx tokens per batch
QT = 4096          # queries per batch
DEPTH = 4
FF = 4096          # w1 out
FH = 2048          # geglu hidden
KT = D // 128      # 4 feature tiles
EPS = 1e-5
TWO_PI = 2.0 * math.pi
MAGIC = float(1.5 * 2 ** 23)  # fp32 round-to-nearest-integer trick

bf16 = ml_dtypes.bfloat16


class Ker:
    """Builds the per-core Bass program."""

    def __init__(self, dbg=False):
        self.dbg = dbg
        nc = bacc.Bacc("TRN2", target_bir_lowering=False, debug=False)
        self.nc = nc
        self.di = {}

        def inp(name, shape, dt):
            self.di[name] = nc.dram_tensor(name, shape, dt, kind="ExternalInput")

        inp("pts_ctx", (B, CT, 3), F32)
        inp("vals_ctx", (B, CT), F32)
        inp("pts_q", (B, QT, 3), F32)
        inp("alpha", (B,), F32)
        inp("freqs", (256,), F32)
        inp("basis", (3, 24), F32)
        inp("w_pe", (128, D), BF16)
        inp("ctx_bias", (D,), F32)
        inp("qe_bias", (D,), F32)
        inp("lat", (LAT, D), F32)
        for pre in ("ca", "sa"):
            inp(pre + "_wq", (DEPTH, D, D), BF16)
            inp(pre + "_wkv", (DEPTH, D, 2 * D), BF16)
            inp(pre + "_wo", (DEPTH, D, D), BF16)
            inp(pre + "_bo", (DEPTH, D), F32)
        for pre in ("cf", "sf"):
            inp(pre + "_w1", (DEPTH, D, FF), BF16)
            inp(pre + "_b1", (DEPTH, FF), F32)
            inp(pre + "_w2", (DEPTH, FH, D), BF16)
            inp(pre + "_b2", (DEPTH, D), F32)
        inp("w_film", (D, 18 * D), BF16)
        inp("ln_static", (D, 28), F32)
        inp("dec_wq", (D, D), BF16)
        inp("dec_wkv", (D, 2 * D), BF16)
        inp("dec_wo", (D, D), BF16)
        inp("dec_bo", (D,), F32)
        inp("out_w", (D,), BF16)
        inp("out_b", (1,), F32)
        inp("zerob", (B * QT,), BF16)

        self.out_dram = nc.dram_tensor("out", (B, QT), F32, kind="ExternalOutput")
        self.dbg_drams = {}
        if dbg:
            for nm, shape, dt in [
                ("dbg_embc", (128, B * CT), BF16),
                ("dbg_film", (128, 72, B), F32),
                ("dbg_label", (128, 4, B), BF16),
                ("dbg_x0", (128, 8, D), F32), ("dbg_x1", (128, 8, D), F32),
                ("dbg_x2", (128, 8, D), F32), ("dbg_x3", (128, 8, D), F32),
                ("dbg_x4", (128, 8, D), F32), ("dbg_xf", (128, 8, D), F32),
                ("dbg_xnT", (KT, 128, T), BF16), ("dbg_cnT", (KT, 128, T), BF16),
                ("dbg_ctx0", (8, 128, D), F32), ("dbg_embc2", (128, B * CT), BF16),
                ("dbg_qT", (KT, 128, T), BF16), ("dbg_kT", (KT, 128, T), BF16),
                ("dbg_vT", (KT, 128, T), BF16), ("dbg_yT", (KT, 128, T), BF16),
            ]:
                self.dbg_drams[nm] = nc.dram_tensor(nm, shape, dt, kind="ExternalOutput")

        self.ev_ctr = 0
        with ExitStack() as ctx:
            self.ctx = ctx
            self.tc = ctx.enter_context(tile.TileContext(nc))
            self.build()
        nc.finalize()

    # ------------------------------------------------------------------
    def evict(self, out_ap, in_ap):
        self.ev_ctr += 1
        if self.ev_ctr % 2 == 0:
            self.nc.scalar.copy(out_ap, in_ap)
        else:
            self.nc.vector.tensor_copy(out_ap, in_ap)

    def evict_affine(self, out_ap, in_ap, S_ap, B_ap):
        self.ev_ctr += 1
        if self.ev_ctr % 2 == 0:
            self.nc.scalar.activation(out_ap, in_ap, AF.Identity, bias=B_ap, scale=S_ap)
        else:
            self.nc.vector.tensor_scalar(out=out_ap, in0=in_ap, scalar1=S_ap,
                                         scalar2=B_ap, op0=ALU.mult, op1=ALU.add)

    def evict_bias(self, out_ap, in_ap, B_ap):
        self.ev_ctr += 1
        if self.ev_ctr % 2 == 0:
            self.nc.scalar.activation(out_ap, in_ap, AF.Identity, bias=B_ap)
        else:
            self.nc.vector.tensor_scalar_add(out_ap, in_ap, B_ap)

    def tap(self, name, ap):
        if self.dbg and name in self.dbg_drams:
            self.nc.sync.dma_start(self.dbg_drams[name].ap(), ap)

    def tap_tiles(self, name, tiles):
        if self.dbg and name in self.dbg_drams:
            for i, t_ in enumerate(tiles):
                self.nc.sync.dma_start(self.dbg_drams[name].ap()[i], t_[:])

    # ------------------------------------------------------------------
    def build(self):
        nc, tc, ctx, di = self.nc, self.tc, self.ctx, self.di

        # ---- pools (sized to stay under SBUF budget; see design notes) ----
        self.consts = ctx.enter_context(tc.tile_pool(name="consts", bufs=1))
        self.wp512 = ctx.enter_context(tc.tile_pool(name="wp512", bufs=18))
        self.wp1024 = ctx.enter_context(tc.tile_pool(name="wp1024", bufs=5))
        self.actT = ctx.enter_context(tc.tile_pool(name="actT", bufs=24))
        self.ep = ctx.enter_context(tc.tile_pool(name="ep", bufs=6))
        self.xhp = ctx.enter_context(tc.tile_pool(name="xhp", bufs=8))
        self.svp = ctx.enter_context(tc.tile_pool(name="svp", bufs=16))
        self.sv6p = ctx.enter_context(tc.tile_pool(name="sv6p", bufs=4))
        self.sv2p = ctx.enter_context(tc.tile_pool(name="sv2p", bufs=4))
        self.colp = ctx.enter_context(tc.tile_pool(name="colp", bufs=4))
        self.ps = ctx.enter_context(tc.tile_pool(name="ps", bufs=5, space="PSUM"))
        self.pst = ctx.enter_context(tc.tile_pool(name="pst", bufs=3, space="PSUM"))
        self.psf = self.ps

        consts, svp, colp = self.consts, self.svp, self.colp

        # ---- constants ----
        self.ident = consts.tile([128, 128], BF16, tag="ident")
        make_identity(nc, self.ident[:])
        self.eps_t = consts.tile([128, 1], F32, tag="eps")
        nc.vector.memset(self.eps_t[:], EPS)
        self.negpi = consts.tile([128, 1], F32, tag="negpi")
        nc.vector.memset(self.negpi[:], -math.pi)
        self.w_pe_sb = consts.tile([128, D], BF16, tag="w_pe")
        nc.sync.dma_start(self.w_pe_sb[:], di["w_pe"].ap())
        self.basis_sb = consts.tile([3, 24], F32, tag="basis")
        nc.sync.dma_start(self.basis_sb[:], di["basis"].ap())

        self.lns = []
        for kt in range(KT):
            t_ = consts.tile([128, 28], F32, tag=f"lns{kt}")
            nc.sync.dma_start(t_[:], di["ln_static"].ap()[kt * 128:(kt + 1) * 128, :])
            self.lns.append(t_)

        def bias_bcast(name):
            t_ = consts.tile([128, D], F32, tag=f"bb_{name}")
            src = di[name].ap()
            ap = bass.AP(tensor=src.tensor, offset=src.offset,
                         ap=[[0, 128]] + list(src.ap))
            nc.sync.dma_start(t_[:], ap)
            return t_

        self.ctx_bias_bc = bias_bcast("ctx_bias")
        self.qe_bias_bc = bias_bcast("qe_bias")

        # residual stream x: [128, 8 token-tiles, 512] fp32
        self.x = consts.tile([128, 8, D], F32, tag="x")
        lat_ap = di["lat"].ap()
        for tt in range(8):
            r = (tt % 4) * 128
            nc.sync.dma_start(self.x[:, tt, :], lat_ap[r:r + 128, :])

        # ---- label + film vectors ----
        freq_sb = consts.tile([128, 2], F32, tag="freqs")
        nc.sync.dma_start(freq_sb[:], di["freqs"].ap().rearrange("(c p) -> p c", p=128))
        al_src = di["alpha"].ap()
        alpha_bc = consts.tile([128, B], F32, tag="alpha")
        nc.sync.dma_start(
            alpha_bc[:],
            bass.AP(tensor=al_src.tensor, offset=al_src.offset,
                    ap=[[0, 128]] + list(al_src.ap)))

        labelT = consts.tile([128, 4, B], BF16, tag="labelT")
        for b in range(B):
            t_ = colp.tile([128, 2], F32, tag="lbl_t")
            nc.vector.tensor_scalar_mul(t_[:], freq_sb[:], alpha_bc[:, b:b + 1])
            u_ = colp.tile([128, 2], F32, tag="lbl_u")
            nc.vector.tensor_scalar_add(u_[:], t_[:], 0.25)
            k_ = colp.tile([128, 2], F32, tag="lbl_k")
            nc.vector.tensor_scalar(out=k_[:], in0=t_[:], scalar1=MAGIC, scalar2=MAGIC,
                                    op0=ALU.add, op1=ALU.subtract)
            nc.vector.tensor_sub(t_[:], t_[:], k_[:])
            nc.vector.tensor_scalar(out=k_[:], in0=u_[:], scalar1=MAGIC, scalar2=MAGIC,
                                    op0=ALU.add, op1=ALU.subtract)
            nc.vector.tensor_sub(u_[:], u_[:], k_[:])
            nc.scalar.activation(labelT[:, 0:2, b], u_[:], AF.Sin, scale=TWO_PI)
            nc.scalar.activation(labelT[:, 2:4, b], t_[:], AF.Sin, scale=TWO_PI)

        # film_vecs[p, 72, b] fp32; vec idx = matrix_idx*4 + ft
        self.film = consts.tile([128, 72, B], F32, tag="film")
        wf = di["w_film"].ap()
        for mt in range(72):
            pf = self.psf.tile([128, 512], F32, tag="ps")
            for kt in range(KT):
                wt = self.wp512.tile([128, 128], BF16, tag="wfilm")
                nc.sync.dma_start(wt[:], wf[kt * 128:(kt + 1) * 128,
                                             mt * 128:(mt + 1) * 128])
                nc.tensor.matmul(pf[:, 0:B], wt[:], labelT[:, kt, :],
                                 start=(kt == 0), stop=(kt == KT - 1))
            self.evict(self.film[:, mt, :], pf[:, 0:B])

        # ---- context point-embed table ----
        self.embT_ctx = consts.tile([128, B * CT], BF16, tag="embc")
        self.build_embT(self.embT_ctx, di["pts_ctx"], B * CT, CT, di["vals_ctx"])

        self.tap("dbg_embc", self.embT_ctx[:])
        self.tap("dbg_film", self.film[:])
        self.tap("dbg_label", labelT[:])
        self.tap("dbg_x0", self.x[:])

        # ---- layers ----
        for l in range(DEPTH):
            self.layer(l)

        self.tap("dbg_xf", self.x[:])

        # ---- decode ----
        self.decode()

    # ------------------------------------------------------------------
    def build_embT(self, embT, pts_dram, n_tok, per_b, vals_dram=None):
        """Fill embT [128, n_tok] bf16: rows 0-23 -sin, 32-55 -cos, 64-66 pts, 96 vals/0."""
        nc = self.nc
        with self.tc.tile_pool(name="pep", bufs=2) as pep, \
             self.tc.tile_pool(name="fop", bufs=2) as fop:
            self._embT_body(embT, pts_dram, n_tok, per_b, vals_dram, pep, fop)

    def _embT_body(self, embT, pts_dram, n_tok, per_b, vals_dram, pep, fop):
        nc = self.nc
        self.pep, self.fop = pep, fop
        zsrc = self.di["zerob"].ap()
        nc.sync.dma_start(embT[:], bass.AP(tensor=zsrc.tensor, offset=zsrc.offset,
                                           ap=[[0, 128], [1, n_tok]]))
        for ch in range(n_tok // 512):
            b = (ch * 512) // per_b
            t0 = (ch * 512) % per_b
            ptsT = self.pep.tile([3, 512], F32, tag="ptsT")
            for d3 in range(3):
                nc.sync.dma_start(ptsT[d3:d3 + 1, :],
                                  pts_dram.ap()[b, t0:t0 + 512, d3:d3 + 1])
            pp = self.ps.tile([128, 512], F32, tag="ps")
            nc.tensor.matmul(pp[0:24, :], self.basis_sb[:], ptsT[:],
                             start=True, stop=True)
            proj = self.pep.tile([24, 512], F32, tag="proj")
            self.evict(proj[:], pp[0:24, :])
            fc = self.pep.tile([24, 512], F32, tag="frac_c")
            nc.vector.tensor_scalar_add(fc[:], proj[:], 0.25)
            k_ = self.pep.tile([24, 512], F32, tag="kr")
            nc.vector.tensor_scalar(out=k_[:], in0=proj[:], scalar1=MAGIC, scalar2=MAGIC,
                                    op0=ALU.add, op1=ALU.subtract)
            nc.vector.tensor_sub(proj[:], proj[:], k_[:])
            nc.vector.tensor_scalar(out=k_[:], in0=fc[:], scalar1=MAGIC, scalar2=MAGIC,
                                    op0=ALU.add, op1=ALU.subtract)
            nc.vector.tensor_sub(fc[:], fc[:], k_[:])
            sl = slice(ch * 512, ch * 512 + 512)
            nc.scalar.activation(embT[0:24, sl], proj[:], AF.Sin, scale=TWO_PI)
            nc.scalar.activation(embT[32:56, sl], fc[:], AF.Sin, scale=TWO_PI)
            nc.vector.tensor_copy(embT[64:67, sl], ptsT[:])
            if vals_dram is not None:
                vch = self.fop.tile([1, 512], F32, tag="vch")
                nc.sync.dma_start(vch[:], vals_dram.ap()[b, t0:t0 + 512])
                nc.vector.tensor_copy(embT[96:97, sl], vch[:])
        if vals_dram is None:
            nc.vector.memset(embT[96:97, :], 0.0)

    # ------------------------------------------------------------------
    def ln_to_T(self, src_fn, n_tt, S_fn, B_fn):
        """LN token-major tiles -> feature-major bf16 tiles with per-feature affine."""
        nc = self.nc
        xh = []
        for tt in range(n_tt):
            xt = src_fn(tt)
            stats = self.sv6p.tile([128, 6], F32, tag="stats")
            nc.vector.bn_stats(stats[:], xt)
            mv = self.sv2p.tile([128, 2], F32, tag="mv")
            nc.vector.bn_aggr(mv[:], stats[:])
            std = self.svp.tile([128, 1], F32, tag="std")
            nc.scalar.activation(std[:], mv[:, 1:2], AF.Sqrt, bias=self.eps_t[:, 0:1])
            rstd = self.svp.tile([128, 1], F32, tag="rstd")
            nc.vector.reciprocal(rstd[:], std[:])
            nmr = self.svp.tile([128, 1], F32, tag="nmr")
            nc.vector.tensor_scalar(out=nmr[:], in0=mv[:, 0:1], scalar1=rstd[:, 0:1],
                                    scalar2=-1.0, op0=ALU.mult, op1=ALU.mult)
            xh_t = self.xhp.tile([128, 512], BF16, tag="xh")
            self.evict_affine(xh_t[:], xt, rstd[:, 0:1], nmr[:, 0:1])
            xh.append(xh_t)
        outs = []
        for ft in range(KT):
            o = self.actT.tile([128, n_tt * 128], BF16, tag="actT")
            for q in range(n_tt // 4):
                pq = self.pst.tile([128, 4, 128], BF16, tag="pst")
                for j in range(4):
                    nc.tensor.transpose(pq[:, j, :],
                                        xh[q * 4 + j][:, ft * 128:(ft + 1) * 128],
                                        self.ident[:])
                self.evict_affine(o[:, q * 512:(q + 1) * 512], pq[:],
                                  S_fn(ft, q), B_fn(ft, q))
            outs.append(o)
        return outs

    # ------------------------------------------------------------------
    def mm_to_T(self, w_tiles, rhsT, n_mt, n_cols, bias_fn=None):
        nc = self.nc
        outs = []
        for mt in range(n_mt):
            o = self.actT.tile([128, n_cols], BF16, tag="actT")
            for h in range(n_cols // 512):
                pp = self.ps.tile([128, 512], F32, tag="ps")
                nk = len(w_tiles)
                for kt in range(nk):
                    nc.tensor.matmul(pp[:], w_tiles[kt][:, mt * 128:(mt + 1) * 128],
                                     rhsT[kt][:, h * 512:(h + 1) * 512],
                                     start=(kt == 0), stop=(kt == nk - 1))
                if bias_fn is None:
                    self.evict(o[:, h * 512:(h + 1) * 512], pp[:])
                else:
                    self.evict_bias(o[:, h * 512:(h + 1) * 512], pp[:], bias_fn(mt))
            outs.append(o)
        return outs

    def load_w(self, dram_ap, n_kt, width, tag, pool):
        tiles = []
        for kt in range(n_kt):
            w = pool.tile([128, width], BF16, tag=tag)
            self.nc.sync.dma_start(w[:], dram_ap[kt * 128:(kt + 1) * 128, :])
            tiles.append(w)
        return tiles

    def load_cols(self, dram_ap, n_cols, dt=F32):
        t_ = self.colp.tile([128, n_cols], dt, tag=f"cols{n_cols}")
        self.nc.sync.dma_start(t_[:], dram_ap.rearrange("(c p) -> p c", p=128))
        return t_

    def transpose_V(self, vT, n_tt, vpool):
        nc = self.nc
        V = vpool.tile([128, n_tt, D], BF16, tag="V")
        for tt in range(n_tt):
            pq = self.pst.tile([128, 4, 128], BF16, tag="pst")
            for ft in range(KT):
                nc.tensor.transpose(pq[:, ft, :], vT[ft][:, tt * 128:(tt + 1) * 128],
                                    self.ident[:])
            self.evict(V[:, tt, :], pq[:])
        return V

    def add_residual(self, yT, n_tt):
        nc = self.nc
        for tt in range(n_tt):
            pq = self.pst.tile([128, 4, 128], BF16, tag="pst")
            for ft in range(KT):
                nc.tensor.transpose(pq[:, ft, :], yT[ft][:, tt * 128:(tt + 1) * 128],
                                    self.ident[:])
            nc.vector.tensor_tensor(out=self.x[:, tt, :], in0=self.x[:, tt, :],
                                    in1=pq[:], op=ALU.add)

    def softmax_rows(self, pp):
        # scores are bounded (|s| < ~4) so no max-subtraction is needed
        nc = self.nc
        sums = self.svp.tile([128, 1], F32, tag="sums")
        E = self.ep.tile([128, 512], BF16, tag="E")
        nc.scalar.activation(E[:], pp[:], AF.Exp, accum_out=sums[:])
        rec = self.svp.tile([128, 1], F32, tag="rec")
        nc.vector.reciprocal(rec[:], sums[:])
        A = self.ep.tile([128, 512], BF16, tag="E")
        nc.vector.tensor_scalar_mul(A[:], E[:], rec[:, 0:1])
        return A

    # ------------------------------------------------------------------
    def attention(self, qT, kT, vT, wo_tiles, bo_c):
        nc, tc = self.nc, self.tc
        with tc.tile_pool(name="vpool", bufs=1) as vpool, \
             tc.tile_pool(name="atp", bufs=8) as atp:
            V = self.transpose_V(vT, 8, vpool)
            OT = [self.actT.tile([128, T], BF16, tag="actT", name=f"OT{i}")
                  for i in range(KT)]
            for b in range(B):
                ps_o = None
                for h in range(HEADS):
                    ft, sub = h // 2, h % 2
                    po = sub * 64
                    Es = []
                    for qt in range(4):
                        pp = self.ps.tile([128, 512], F32, tag="ps")
                        nc.tensor.matmul(
                            pp[:],
                            qT[ft][po:po + 64,
                                   b * 512 + qt * 128: b * 512 + (qt + 1) * 128],
                            kT[ft][po:po + 64, b * 512:(b + 1) * 512],
                            start=True, stop=True)
                        Es.append(self.softmax_rows(pp))
                    ATt = []
                    for kt in range(4):
                        pq = self.pst.tile([128, 4, 128], BF16, tag="pst")
                        for qt in range(4):
                            nc.tensor.transpose(pq[:, qt, :],
                                                Es[qt][:, kt * 128:(kt + 1) * 128],
                                                self.ident[:])
                        at = atp.tile([128, 512], BF16, tag="AT")
                        self.evict(at[:], pq[:])
                        ATt.append(at)
                    if sub == 0:
                        ps_o = self.ps.tile([128, 512], F32, tag="ps")
                    for jt in range(4):
                        nc.tensor.matmul(ps_o[po:po + 64, :],
                                         V[:, b * 4 + jt, h * 64:(h + 1) * 64],
                                         ATt[jt][:], start=(jt == 0), stop=(jt == 3))
                    if sub == 1:
                        self.evict(OT[ft][:, b * 512:(b + 1) * 512], ps_o[:])
            return self.mm_to_T(wo_tiles, OT, KT, T,
                                bias_fn=lambda mt: bo_c[:, mt:mt + 1])

    # ------------------------------------------------------------------
    def ffn(self, xnT, l, pre):
        nc, tc, di = self.nc, self.tc, self.di
        with tc.tile_pool(name=f"w4k_{pre}{l}", bufs=4) as wp4096, \
             tc.tile_pool(name=f"ag_{pre}{l}", bufs=16) as agp:
            w1 = self.load_w(di[pre + "_w1"].ap()[l], KT, FF, "w1", wp4096)
            b1c = self.load_cols(di[pre + "_b1"].ap()[l], 32)
            ag = []
            for f in range(16):
                o = agp.tile([128, T], BF16, tag="ag")
                for h in range(2):
                    pa = self.ps.tile([128, 512], F32, tag="ps")
                    pg = self.ps.tile([128, 512], F32, tag="ps")
                    for kt in range(KT):
                        nc.tensor.matmul(pa[:], w1[kt][:, f * 128:(f + 1) * 128],
                                         xnT[kt][:, h * 512:(h + 1) * 512],
                                         start=(kt == 0), stop=(kt == KT - 1))
                    for kt in range(KT):
                        nc.tensor.matmul(pg[:], w1[kt][:, (16 + f) * 128:(17 + f) * 128],
                                         xnT[kt][:, h * 512:(h + 1) * 512],
                                         start=(kt == 0), stop=(kt == KT - 1))
                    a_sb = self.ep.tile([128, 512], BF16, tag="E")
                    self.evict_bias(a_sb[:], pa[:], b1c[:, f:f + 1])
                    g_sb = self.ep.tile([128, 512], BF16, tag="E")
                    nc.scalar.activation(g_sb[:], pg[:], AF.Gelu,
                                         bias=b1c[:, 16 + f:17 + f])
                    nc.vector.tensor_mul(o[:, h * 512:(h + 1) * 512], a_sb[:], g_sb[:])
                ag.append(o)
            w2 = self.load_w(di[pre + "_w2"].ap()[l], FH // 128, D, "w512", self.wp512)
            b2c = self.load_cols(di[pre + "_b2"].ap()[l], 4)
            yT = self.mm_to_T(w2, ag, KT, T, bias_fn=lambda mt: b2c[:, mt:mt + 1])
            self.add_residual(yT, 8)

    # ------------------------------------------------------------------
    def layer(self, l):
        nc, di = self.nc, self.di
        lns = self.lns
        c0 = l * 6
        x_tile = lambda tt: self.x[:, tt, :]

        # cross-attention
        xnT = self.ln_to_T(x_tile, 8,
                           lambda ft, q: lns[ft][:, c0 + 0:c0 + 1],
                           lambda ft, q: lns[ft][:, c0 + 1:c0 + 2])
        with self.tc.tile_pool(name=f"ctxp{l}", bufs=8) as ctxp:
            ctx_tiles = []
            for tt in range(8):
                b, i = tt // 4, tt % 4
                col = b * CT + l * 512 + i * 128
                pp = self.ps.tile([128, 512], F32, tag="ps")
                nc.tensor.matmul(pp[:], self.embT_ctx[:, col:col + 128],
                                 self.w_pe_sb[:], start=True, stop=True)
                ct = ctxp.tile([128, 512], F32, tag="f32t")
                nc.vector.tensor_tensor(out=ct[:], in0=pp[:], in1=self.ctx_bias_bc[:],
                                        op=ALU.add)
                ctx_tiles.append(ct)
            if l == 0:
                self.tap("dbg_embc2", self.embT_ctx[:])
                self.tap_tiles("dbg_ctx0", ctx_tiles)
            cnT = self.ln_to_T(lambda tt: ctx_tiles[tt][:], 8,
                               lambda ft, q: lns[ft][:, c0 + 2:c0 + 3],
                               lambda ft, q: lns[ft][:, c0 + 3:c0 + 4])
        wq = self.load_w(di["ca_wq"].ap()[l], KT, D, "w512", self.wp512)
        wkv = self.load_w(di["ca_wkv"].ap()[l], KT, 2 * D, "wkv", self.wp1024)
        wo = self.load_w(di["ca_wo"].ap()[l], KT, D, "w512", self.wp512)
        bo_c = self.load_cols(di["ca_bo"].ap()[l], 4)
        qT = self.mm_to_T(wq, xnT, KT, T)
        kvT = self.mm_to_T(wkv, cnT, 2 * KT, T)
        if l == 0:
            self.tap_tiles("dbg_xnT", xnT)
            self.tap_tiles("dbg_cnT", cnT)
            self.tap_tiles("dbg_qT", qT)
            self.tap_tiles("dbg_kT", kvT[:KT])
            self.tap_tiles("dbg_vT", kvT[KT:])
        yT = self.attention(qT, kvT[:KT], kvT[KT:], wo, bo_c)
        if l == 0:
            self.tap_tiles("dbg_yT", yT)
        self.add_residual(yT, 8)
        if l == 0:
            self.tap("dbg_x1", self.x[:])

        # cross FFN
        fnT = self.ln_to_T(x_tile, 8,
                           lambda ft, q: lns[ft][:, c0 + 4:c0 + 5],
                           lambda ft, q: lns[ft][:, c0 + 5:c0 + 6])
        self.ffn(fnT, l, "cf")
        if l == 0:
            self.tap("dbg_x2", self.x[:])

        # FiLM self-attention (film vec idx base: l*16; q index = batch here)
        film = self.film
        sa_s, sa_b = l * 16 + 0, l * 16 + 4
        snT = self.ln_to_T(x_tile, 8,
                           lambda ft, q: film[:, sa_s + ft, q:q + 1],
                           lambda ft, q: film[:, sa_b + ft, q:q + 1])
        wq = self.load_w(di["sa_wq"].ap()[l], KT, D, "w512", self.wp512)
        wkv = self.load_w(di["sa_wkv"].ap()[l], KT, 2 * D, "wkv", self.wp1024)
        wo = self.load_w(di["sa_wo"].ap()[l], KT, D, "w512", self.wp512)
        bo_c = self.load_cols(di["sa_bo"].ap()[l], 4)
        qT = self.mm_to_T(wq, snT, KT, T)
        kvT = self.mm_to_T(wkv, snT, 2 * KT, T)
        yT = self.attention(qT, kvT[:KT], kvT[KT:], wo, bo_c)
        self.add_residual(yT, 8)
        if l == 0:
            self.tap("dbg_x3", self.x[:])

        # FiLM FFN
        sf_s, sf_b = l * 16 + 8, l * 16 + 12
        snT = self.ln_to_T(x_tile, 8,
                           lambda ft, q: film[:, sf_s + ft, q:q + 1],
                           lambda ft, q: film[:, sf_b + ft, q:q + 1])
        self.ffn(snT, l, "sf")
        if l == 0:
            self.tap("dbg_x4", self.x[:])

    # ------------------------------------------------------------------
    def decode(self):
        nc, tc, di = self.nc, self.tc, self.di
        lns, film = self.lns, self.film
        x_tile = lambda tt: self.x[:, tt, :]

        cnT = self.ln_to_T(x_tile, 8,
                           lambda ft, q: lns[ft][:, 26:27],
                           lambda ft, q: lns[ft][:, 27:28])
        dwkv = self.load_w(di["dec_wkv"].ap(), KT, 2 * D, "wkv", self.wp1024)
        kvT = self.mm_to_T(dwkv, cnT, 2 * KT, T)
        kTd, vTd = kvT[:KT], kvT[KT:]
        dwq = self.load_w(di["dec_wq"].ap(), KT, D, "w512", self.wp512)
        dwo = self.load_w(di["dec_wo"].ap(), KT, D, "w512", self.wp512)
        dbo_c = self.load_cols(di["dec_bo"].ap(), 4)
        outw_c = self.load_cols(di["out_w"].ap(), 4, dt=BF16)
        outb_sb = self.consts.tile([1, 1], F32, tag="outb")
        nc.sync.dma_start(outb_sb[:], di["out_b"].ap())

        out_view = self.out_dram.ap().rearrange("b (c t) -> b c t", t=512)

        with tc.tile_pool(name="vpool_d", bufs=1) as vpool, \
             tc.tile_pool(name="atp_d", bufs=8) as atp, \
             tc.tile_pool(name="qep", bufs=6) as qep, \
             tc.tile_pool(name="embq", bufs=1) as embqp:
            Vd = self.transpose_V(vTd, 8, vpool)
            embT_q = embqp.tile([128, B * QT], BF16, tag="embq")
            self.build_embT(embT_q, di["pts_q"], B * QT, QT)

            for qc in range(16):
                b = qc // 8
                qhat = []
                for i in range(4):
                    col = qc * 512 + i * 128
                    pp = self.ps.tile([128, 512], F32, tag="ps")
                    nc.tensor.matmul(pp[:], embT_q[:, col:col + 128], self.w_pe_sb[:],
                                     start=True, stop=True)
                    qe = qep.tile([128, 512], F32, tag="qe")
                    nc.vector.tensor_tensor(out=qe[:], in0=pp[:],
                                            in1=self.qe_bias_bc[:], op=ALU.add)
                    qhat.append(qe)
                qnT = self.ln_to_T(lambda i: qhat[i][:], 4,
                                   lambda ft, q: lns[ft][:, 24:25],
                                   lambda ft, q: lns[ft][:, 25:26])
                qTt = self.mm_to_T(dwq, qnT, KT, 512)
                Es = []
                for i in range(4):
                    pp = self.ps.tile([128, 512], F32, tag="ps")
                    for kt in range(KT):
                        nc.tensor.matmul(pp[:], qTt[kt][:, i * 128:(i + 1) * 128],
                                         kTd[kt][:, b * 512:(b + 1) * 512],
                                         start=(kt == 0), stop=(kt == KT - 1))
                    Es.append(self.softmax_rows(pp))
                ATt = []
                for jt in range(4):
                    pq = self.pst.tile([128, 4, 128], BF16, tag="pst")
                    for i in range(4):
                        nc.tensor.transpose(pq[:, i, :], Es[i][:, jt * 128:(jt + 1) * 128],
                                            self.ident[:])
                    at = atp.tile([128, 512], BF16, tag="AT")
                    self.evict(at[:], pq[:])
                    ATt.append(at)
                OTt = []
                for ft in range(KT):
                    po = self.ps.tile([128, 512], F32, tag="ps")
                    for jt in range(4):
                        nc.tensor.matmul(po[:],
                                         Vd[:, b * 4 + jt, ft * 128:(ft + 1) * 128],
                                         ATt[jt][:], start=(jt == 0), stop=(jt == 3))
                    ot = self.actT.tile([128, 512], BF16, tag="actT")
                    self.evict(ot[:], po[:])
                    OTt.append(ot)
                yTt = self.mm_to_T(dwo, OTt, KT, 512,
                                   bias_fn=lambda mt: dbo_c[:, mt:mt + 1])
                ohat = []
                for i in range(4):
                    pq = self.pst.tile([128, 4, 128], BF16, tag="pst")
                    for ft in range(KT):
                        nc.tensor.transpose(pq[:, ft, :], yTt[ft][:, i * 128:(i + 1) * 128],
                                            self.ident[:])
                    o_sb = qep.tile([128, 512], F32, tag="qe")
                    self.evict(o_sb[:], pq[:])
                    ohat.append(o_sb)
                onT = self.ln_to_T(lambda i: ohat[i][:], 4,
                                   lambda ft, q: film[:, 64 + ft, b:b + 1],
                                   lambda ft, q: film[:, 68 + ft, b:b + 1])
                pf = self.psf.tile([128, 512], F32, tag="ps")
                for kt in range(KT):
                    nc.tensor.matmul(pf[0:1, :], outw_c[:, kt:kt + 1], onT[kt][:],
                                     start=(kt == 0), stop=(kt == KT - 1))
                fo = qep.tile([1, 512], F32, tag="fo")
                nc.scalar.activation(fo[:], pf[0:1, :], AF.Identity,
                                     bias=outb_sb[:, 0:1])
                nc.sync.dma_start(out_view[b, qc % 8, :], fo[:])


# ---------------------------------------------------------------------------
# host side
# ---------------------------------------------------------------------------

_NC_CACHE = None


def _get_nc():
    global _NC_CACHE
    if _NC_CACHE is None:
        _NC_CACHE = Ker().nc
    return _NC_CACHE


def _prep_params(params):
    P = {k: np.asarray(v, dtype=np.float32) for k, v in params.items() if k != "layers"}
    L = {k: np.asarray(v, dtype=np.float32) for k, v in params["layers"].items()}
    d = {}
    d["freqs"] = P["freqs"]
    e = np.power(2.0, np.arange(8)).astype(np.float32) * np.pi
    basis = np.zeros((3, 24), np.float32)
    basis[0, :8] = e
    basis[1, 8:16] = e
    basis[2, 16:] = e
    d["basis"] = basis / TWO_PI
    pe_w = P["pe_w"]
    w128 = np.zeros((128, D), np.float32)
    w128[0:24] = pe_w[0:24]
    w128[32:56] = pe_w[24:48]
    w128[64:67] = pe_w[48:51]
    w128[96:97] = P["ve_w"]
    d["w_pe"] = w128.astype(bf16)
    d["ctx_bias"] = P["pe_b"] + P["ve_b"]
    d["qe_bias"] = P["pe_b"]
    d["lat"] = P["latent"]
    scale = (D // HEADS) ** -0.5
    for pre in ("ca", "sa"):
        d[pre + "_wq"] = (L[pre + "_wq"] * scale).astype(bf16)
        d[pre + "_wkv"] = L[pre + "_wkv"].astype(bf16)
        d[pre + "_wo"] = L[pre + "_wo"].astype(bf16)
        d[pre + "_bo"] = L[pre + "_bo"]
    for pre in ("cf", "sf"):
        d[pre + "_w1"] = L[pre + "_w1"].astype(bf16)
        d[pre + "_b1"] = L[pre + "_b1"]
        d[pre + "_w2"] = L[pre + "_w2"].astype(bf16)
        d[pre + "_b2"] = L[pre + "_b2"]
    mats = []
    for l in range(DEPTH):
        mats.append(L["sa_g"][l] * L["sa_ln_s"][l][None, :])
        mats.append(L["sa_g"][l] * L["sa_ln_b"][l][None, :] + L["sa_be"][l])
        mats.append(L["sf_g"][l] * L["sf_ln_s"][l][None, :])
        mats.append(L["sf_g"][l] * L["sf_ln_b"][l][None, :] + L["sf_be"][l])
    mats.append(P["out_g"] * P["out_ln_s"][None, :])
    mats.append(P["out_g"] * P["out_ln_b"][None, :] + P["out_be"])
    d["w_film"] = np.concatenate(mats, axis=1).astype(bf16)
    cols = []
    for l in range(DEPTH):
        cols += [L["ca_ln_s"][l], L["ca_ln_b"][l], L["ca_lnc_s"][l], L["ca_lnc_b"][l],
                 L["cf_ln_s"][l], L["cf_ln_b"][l]]
    cols += [P["dec_ln_s"], P["dec_ln_b"], P["dec_lnc_s"], P["dec_lnc_b"]]
    d["ln_static"] = np.stack(cols, axis=1)
    d["dec_wq"] = (P["dec_wq"] * (D ** -0.5)).astype(bf16)
    d["dec_wkv"] = P["dec_wkv"].astype(bf16)
    d["dec_wo"] = P["dec_wo"].astype(bf16)
    d["dec_bo"] = P["dec_bo"]
    d["out_w"] = P["out_w"][:, 0].astype(bf16)
    d["out_b"] = P["out_b"]
    d["zerob"] = np.zeros((B * QT,), bf16)
    return {k: np.ascontiguousarray(v) for k, v in d.items()}


def kernel(context_points, context_values, queries, alpha, params):
    context_points = np.asarray(context_points, np.float32)
    context_values = np.asarray(context_values, np.float32)
    queries = np.asarray(queries, np.float32)
    alpha = np.asarray(alpha, np.float32)

    nc = _get_nc()
    const = _prep_params(params)

    in_maps = []
    for c in range(NCORES):
        s = slice(c * B, (c + 1) * B)
        m = dict(const)
        m["pts_ctx"] = np.ascontiguousarray(context_points[s])
        m["vals_ctx"] = np.ascontiguousarray(context_values[s, :, 0])
        m["pts_q"] = np.ascontiguousarray(queries[s])
        m["alpha"] = np.ascontiguousarray(alpha[s])
        in_maps.append(m)

    res = bass_utils.run_bass_kernel_spmd(nc, in_maps, core_ids=list(range(NCORES)))
    outs = [res.results[c]["out"].reshape(B, QT, 1) for c in range(NCORES)]
    return np.concatenate(outs, axis=0)


if __name__ == "__main__":
    _get_nc()
    print("built ok, instructions:", len(_NC_CACHE.inst_map))


# revision 28
# speedup vs baseline: 2.3286x; 1.0541x over previous
"""Trainium2 Bass kernel for nn_Network_56427280335153 (perceiver-style dense transformer).

Sharding: data-parallel over batch B=16 across 8 cores (2 batches/core), no collectives.
Layout: token-major fp32 residual stream; feature-major bf16 operands for matmuls
(out = lhsT.T @ rhs with weights as stationary lhsT); PE transposes between layouts.
"""
import os
import sys
import math
from contextlib import ExitStack

for _p in ("/opt/trn_rl_repo", "/root/.axon_site/_ro/trn_rl_repo"):
    if os.path.isdir(_p) and _p not in sys.path:
        sys.path.insert(0, _p)

import numpy as np
import ml_dtypes

import concourse.bass as bass
import concourse.tile as tile
from concourse import bacc, mybir
from concourse import bass_utils
from concourse.masks import make_identity

F32 = mybir.dt.float32
BF16 = mybir.dt.bfloat16
AF = mybir.ActivationFunctionType
ALU = mybir.AluOpType
AX = mybir.AxisListType

NCORES = 8
B = 2              # batches per core
LAT = 512          # latents per batch
D = 512            # model dim
HEADS = 8
T = B * LAT        # stacked latent tokens per core
CT = 2048          # ctx tokens per batch
QT = 4096          # queries per batch
DEPTH = 4
FF = 4096          # w1 out
FH = 2048          # geglu hidden
KT = D // 128      # 4 feature tiles
EPS = 1e-5
TWO_PI = 2.0 * math.pi
MAGIC = float(1.5 * 2 ** 23)  # fp32 round-to-nearest-integer trick

bf16 = ml_dtypes.bfloat16


class Ker:
    """Builds the per-core Bass program."""

    def __init__(self, dbg=False):
        self.dbg = dbg
        nc = bacc.Bacc("TRN2", target_bir_lowering=False, debug=False)
        self.nc = nc
        self.di = {}

        def inp(name, shape, dt):
            self.di[name] = nc.dram_tensor(name, shape, dt, kind="ExternalInput")

        inp("pts_ctx", (B, CT, 3), F32)
        inp("vals_ctx", (B, CT), F32)
        inp("pts_q", (B, QT, 3), F32)
        inp("alpha", (B,), F32)
        inp("freqs", (256,), F32)
        inp("basis", (3, 24), F32)
        inp("w_pe", (128, D), BF16)
        inp("ctx_bias", (D,), F32)
        inp("qe_bias", (D,), F32)
        inp("lat", (LAT, D), F32)
        for pre in ("ca", "sa"):
            inp(pre + "_wq", (DEPTH, D, D), BF16)
            inp(pre + "_wkv", (DEPTH, D, 2 * D), BF16)
            inp(pre + "_wo", (DEPTH, D, D), BF16)
            inp(pre + "_bo", (DEPTH, D), F32)
        for pre in ("cf", "sf"):
            inp(pre + "_w1", (DEPTH, D, FF), BF16)
            inp(pre + "_b1", (DEPTH, FF), F32)
            inp(pre + "_w2", (DEPTH, FH, D), BF16)
            inp(pre + "_b2", (DEPTH, D), F32)
        inp("w_film", (D, 18 * D), BF16)
        inp("ln_static", (D, 28), F32)
        inp("dec_wq", (D, D), BF16)
        inp("dec_wkv", (D, 2 * D), BF16)
        inp("dec_wo", (D, D), BF16)
        inp("dec_bo", (D,), F32)
        inp("out_w", (D,), BF16)
        inp("out_b", (1,), F32)
        inp("zerob", (B * QT,), BF16)

        self.out_dram = nc.dram_tensor("out", (B, QT), F32, kind="ExternalOutput")
        self.dbg_drams = {}
        if dbg:
            for nm, shape, dt in [
                ("dbg_embc", (128, B * CT), BF16),
                ("dbg_film", (128, 72, B), F32),
                ("dbg_label", (128, 4, B), BF16),
                ("dbg_x0", (128, 8, D), F32), ("dbg_x1", (128, 8, D), F32),
                ("dbg_x2", (128, 8, D), F32), ("dbg_x3", (128, 8, D), F32),
                ("dbg_x4", (128, 8, D), F32), ("dbg_xf", (128, 8, D), F32),
                ("dbg_xnT", (KT, 128, T), BF16), ("dbg_cnT", (KT, 128, T), BF16),
                ("dbg_ctx0", (8, 128, D), F32), ("dbg_embc2", (128, B * CT), BF16),
                ("dbg_qT", (KT, 128, T), BF16), ("dbg_kT", (KT, 128, T), BF16),
                ("dbg_vT", (KT, 128, T), BF16), ("dbg_yT", (KT, 128, T), BF16),
            ]:
                self.dbg_drams[nm] = nc.dram_tensor(nm, shape, dt, kind="ExternalOutput")

        self.ev_ctr = 0
        with ExitStack() as ctx:
            self.ctx = ctx
            self.tc = ctx.enter_context(tile.TileContext(nc))
            self.build()
        nc.finalize()

    # ------------------------------------------------------------------
    def evict(self, out_ap, in_ap):
        self.ev_ctr += 1
        if self.ev_ctr % 2 == 0:
            self.nc.scalar.copy(out_ap, in_ap)
        else:
            self.nc.vector.tensor_copy(out_ap, in_ap)

    def evict_affine(self, out_ap, in_ap, S_ap, B_ap):
        self.ev_ctr += 1
        if self.ev_ctr % 2 == 0:
            self.nc.scalar.activation(out_ap, in_ap, AF.Identity, bias=B_ap, scale=S_ap)
        else:
            self.nc.vector.tensor_scalar(out=out_ap, in0=in_ap, scalar1=S_ap,
                                         scalar2=B_ap, op0=ALU.mult, op1=ALU.add)

    def evict_bias(self, out_ap, in_ap, B_ap):
        self.ev_ctr += 1
        if self.ev_ctr % 2 == 0:
            self.nc.scalar.activation(out_ap, in_ap, AF.Identity, bias=B_ap)
        else:
            self.nc.vector.tensor_scalar_add(out_ap, in_ap, B_ap)

    def tap(self, name, ap):
        if self.dbg and name in self.dbg_drams:
            self.nc.sync.dma_start(self.dbg_drams[name].ap(), ap)

    def tap_tiles(self, name, tiles):
        if self.dbg and name in self.dbg_drams:
            for i, t_ in enumerate(tiles):
                self.nc.sync.dma_start(self.dbg_drams[name].ap()[i], t_[:])

    # ------------------------------------------------------------------
    def build(self):
        nc, tc, ctx, di = self.nc, self.tc, self.ctx, self.di

        # ---- pools (sized to stay under SBUF budget; see design notes) ----
        self.consts = ctx.enter_context(tc.tile_pool(name="consts", bufs=1))
        self.wp512 = ctx.enter_context(tc.tile_pool(name="wp512", bufs=18))
        self.wp1024 = ctx.enter_context(tc.tile_pool(name="wp1024", bufs=5))
        self.actT = ctx.enter_context(tc.tile_pool(name="actT", bufs=24))
        self.ep = ctx.enter_context(tc.tile_pool(name="ep", bufs=4))
        self.xhp = ctx.enter_context(tc.tile_pool(name="xhp", bufs=8))
        self.svp = ctx.enter_context(tc.tile_pool(name="svp", bufs=16))
        self.sv6p = ctx.enter_context(tc.tile_pool(name="sv6p", bufs=4))
        self.sv2p = ctx.enter_context(tc.tile_pool(name="sv2p", bufs=4))
        self.colp = ctx.enter_context(tc.tile_pool(name="colp", bufs=4))
        self.ps = ctx.enter_context(tc.tile_pool(name="ps", bufs=5, space="PSUM"))
        self.pst = ctx.enter_context(tc.tile_pool(name="pst", bufs=3, space="PSUM"))
        self.psf = self.ps

        consts, svp, colp = self.consts, self.svp, self.colp

        # ---- constants ----
        self.ident = consts.tile([128, 128], BF16, tag="ident")
        make_identity(nc, self.ident[:])
        self.eps_t = consts.tile([128, 1], F32, tag="eps")
        nc.vector.memset(self.eps_t[:], EPS)
        self.negpi = consts.tile([128, 1], F32, tag="negpi")
        nc.vector.memset(self.negpi[:], -math.pi)
        self.w_pe_sb = consts.tile([128, D], BF16, tag="w_pe")
        nc.sync.dma_start(self.w_pe_sb[:], di["w_pe"].ap())
        self.basis_sb = consts.tile([3, 24], F32, tag="basis")
        nc.sync.dma_start(self.basis_sb[:], di["basis"].ap())

        self.lns = []
        for kt in range(KT):
            t_ = consts.tile([128, 28], F32, tag=f"lns{kt}")
            nc.sync.dma_start(t_[:], di["ln_static"].ap()[kt * 128:(kt + 1) * 128, :])
            self.lns.append(t_)

        def bias_bcast(name):
            t_ = consts.tile([128, D], F32, tag=f"bb_{name}")
            src = di[name].ap()
            ap = bass.AP(tensor=src.tensor, offset=src.offset,
                         ap=[[0, 128]] + list(src.ap))
            nc.sync.dma_start(t_[:], ap)
            return t_

        self.ctx_bias_bc = bias_bcast("ctx_bias")
        self.qe_bias_bc = bias_bcast("qe_bias")

        # residual stream x: [128, 8 token-tiles, 512] fp32
        self.x = consts.tile([128, 8, D], F32, tag="x")
        lat_ap = di["lat"].ap()
        for tt in range(8):
            r = (tt % 4) * 128
            nc.sync.dma_start(self.x[:, tt, :], lat_ap[r:r + 128, :])

        # ---- label + film vectors ----
        freq_sb = consts.tile([128, 2], F32, tag="freqs")
        nc.sync.dma_start(freq_sb[:], di["freqs"].ap().rearrange("(c p) -> p c", p=128))
        al_src = di["alpha"].ap()
        alpha_bc = consts.tile([128, B], F32, tag="alpha")
        nc.sync.dma_start(
            alpha_bc[:],
            bass.AP(tensor=al_src.tensor, offset=al_src.offset,
                    ap=[[0, 128]] + list(al_src.ap)))

        labelT = consts.tile([128, 4, B], BF16, tag="labelT")
        for b in range(B):
            t_ = colp.tile([128, 2], F32, tag="lbl_t")
            nc.vector.tensor_scalar_mul(t_[:], freq_sb[:], alpha_bc[:, b:b + 1])
            u_ = colp.tile([128, 2], F32, tag="lbl_u")
            nc.vector.tensor_scalar_add(u_[:], t_[:], 0.25)
            k_ = colp.tile([128, 2], F32, tag="lbl_k")
            nc.vector.tensor_scalar(out=k_[:], in0=t_[:], scalar1=MAGIC, scalar2=MAGIC,
                                    op0=ALU.add, op1=ALU.subtract)
            nc.vector.tensor_sub(t_[:], t_[:], k_[:])
            nc.vector.tensor_scalar(out=k_[:], in0=u_[:], scalar1=MAGIC, scalar2=MAGIC,
                                    op0=ALU.add, op1=ALU.subtract)
            nc.vector.tensor_sub(u_[:], u_[:], k_[:])
            nc.scalar.activation(labelT[:, 0:2, b], u_[:], AF.Sin, scale=TWO_PI)
            nc.scalar.activation(labelT[:, 2:4, b], t_[:], AF.Sin, scale=TWO_PI)

        # film_vecs[p, 72, b] fp32; vec idx = matrix_idx*4 + ft
        self.film = consts.tile([128, 72, B], F32, tag="film")
        wf = di["w_film"].ap()
        with tc.tile_pool(name="wfilm", bufs=6) as wfp:
            for blk in range(4):  # 18 mt per block
                wts = []
                for kt in range(KT):
                    wt = wfp.tile([128, 18 * 128], BF16, tag="wfb")
                    nc.sync.dma_start(
                        wt[:], wf[kt * 128:(kt + 1) * 128,
                                  blk * 2304:(blk + 1) * 2304])
                    wts.append(wt)
                for m in range(18):
                    mt = blk * 18 + m
                    pf = self.psf.tile([128, 512], F32, tag="ps")
                    for kt in range(KT):
                        nc.tensor.matmul(pf[:, 0:B], wts[kt][:, m * 128:(m + 1) * 128],
                                         labelT[:, kt, :],
                                         start=(kt == 0), stop=(kt == KT - 1))
                    self.evict(self.film[:, mt, :], pf[:, 0:B])

        # ---- context point-embed table ----
        self.embT_ctx = consts.tile([128, B * CT], BF16, tag="embc")
        self.build_embT(self.embT_ctx, di["pts_ctx"], B * CT, CT, di["vals_ctx"])

        self.tap("dbg_embc", self.embT_ctx[:])
        self.tap("dbg_film", self.film[:])
        self.tap("dbg_label", labelT[:])
        self.tap("dbg_x0", self.x[:])

        # ---- layers ----
        for l in range(DEPTH):
            self.layer(l)

        self.tap("dbg_xf", self.x[:])

        # ---- decode ----
        self.decode()

    # ------------------------------------------------------------------
    def build_embT(self, embT, pts_dram, n_tok, per_b, vals_dram=None):
        """Fill embT [128, n_tok] bf16: rows 0-23 -sin, 32-55 -cos, 64-66 pts, 96 vals/0."""
        nc = self.nc
        with self.tc.tile_pool(name="pep", bufs=2) as pep, \
             self.tc.tile_pool(name="fop", bufs=2) as fop:
            self._embT_body(embT, pts_dram, n_tok, per_b, vals_dram, pep, fop)

    def _embT_body(self, embT, pts_dram, n_tok, per_b, vals_dram, pep, fop):
        nc = self.nc
        self.pep, self.fop = pep, fop
        zsrc = self.di["zerob"].ap()
        nc.sync.dma_start(embT[:], bass.AP(tensor=zsrc.tensor, offset=zsrc.offset,
                                           ap=[[0, 128], [1, n_tok]]))
        for ch in range(n_tok // 512):
            b = (ch * 512) // per_b
            t0 = (ch * 512) % per_b
            ptsT = self.pep.tile([3, 512], F32, tag="ptsT")
            for d3 in range(3):
                nc.sync.dma_start(ptsT[d3:d3 + 1, :],
                                  pts_dram.ap()[b, t0:t0 + 512, d3:d3 + 1])
            pp = self.ps.tile([128, 512], F32, tag="ps")
            nc.tensor.matmul(pp[0:24, :], self.basis_sb[:], ptsT[:],
                             start=True, stop=True)
            proj = self.pep.tile([24, 512], F32, tag="proj")
            self.evict(proj[:], pp[0:24, :])
            fc = self.pep.tile([24, 512], F32, tag="frac_c")
            nc.vector.tensor_scalar_add(fc[:], proj[:], 0.25)
            k_ = self.pep.tile([24, 512], F32, tag="kr")
            nc.vector.tensor_scalar(out=k_[:], in0=proj[:], scalar1=MAGIC, scalar2=MAGIC,
                                    op0=ALU.add, op1=ALU.subtract)
            nc.vector.tensor_sub(proj[:], proj[:], k_[:])
            nc.vector.tensor_scalar(out=k_[:], in0=fc[:], scalar1=MAGIC, scalar2=MAGIC,
                                    op0=ALU.add, op1=ALU.subtract)
            nc.vector.tensor_sub(fc[:], fc[:], k_[:])
            sl = slice(ch * 512, ch * 512 + 512)
            nc.scalar.activation(embT[0:24, sl], proj[:], AF.Sin, scale=TWO_PI)
            nc.scalar.activation(embT[32:56, sl], fc[:], AF.Sin, scale=TWO_PI)
            nc.vector.tensor_copy(embT[64:67, sl], ptsT[:])
            if vals_dram is not None:
                vch = self.fop.tile([1, 512], F32, tag="vch")
                nc.sync.dma_start(vch[:], vals_dram.ap()[b, t0:t0 + 512])
                nc.vector.tensor_copy(embT[96:97, sl], vch[:])
        if vals_dram is None:
            nc.vector.memset(embT[96:97, :], 0.0)

    # ------------------------------------------------------------------
    def ln_to_T(self, src_fn, n_tt, S_fn, B_fn):
        """LN token-major tiles -> feature-major bf16 tiles with per-feature affine."""
        nc = self.nc
        xh = []
        for tt in range(n_tt):
            xt = src_fn(tt)
            stats = self.sv6p.tile([128, 6], F32, tag="stats")
            nc.vector.bn_stats(stats[:], xt)
            mv = self.sv2p.tile([128, 2], F32, tag="mv")
            nc.vector.bn_aggr(mv[:], stats[:])
            std = self.svp.tile([128, 1], F32, tag="std")
            nc.scalar.activation(std[:], mv[:, 1:2], AF.Sqrt, bias=self.eps_t[:, 0:1])
            rstd = self.svp.tile([128, 1], F32, tag="rstd")
            nc.vector.reciprocal(rstd[:], std[:])
            nmr = self.svp.tile([128, 1], F32, tag="nmr")
            nc.vector.tensor_scalar(out=nmr[:], in0=mv[:, 0:1], scalar1=rstd[:, 0:1],
                                    scalar2=-1.0, op0=ALU.mult, op1=ALU.mult)
            xh_t = self.xhp.tile([128, 512], BF16, tag="xh")
            self.evict_affine(xh_t[:], xt, rstd[:, 0:1], nmr[:, 0:1])
            xh.append(xh_t)
        outs = []
        for ft in range(KT):
            o = self.actT.tile([128, n_tt * 128], BF16, tag="actT")
            for q in range(n_tt // 4):
                pq = self.pst.tile([128, 4, 128], BF16, tag="pst")
                for j in range(4):
                    nc.tensor.transpose(pq[:, j, :],
                                        xh[q * 4 + j][:, ft * 128:(ft + 1) * 128],
                                        self.ident[:])
                self.evict_affine(o[:, q * 512:(q + 1) * 512], pq[:],
                                  S_fn(ft, q), B_fn(ft, q))
            outs.append(o)
        return outs

    # ------------------------------------------------------------------
    def mm_to_T(self, w_tiles, rhsT, n_mt, n_cols, bias_fn=None):
        nc = self.nc
        outs = []
        for mt in range(n_mt):
            o = self.actT.tile([128, n_cols], BF16, tag="actT")
            for h in range(n_cols // 512):
                pp = self.ps.tile([128, 512], F32, tag="ps")
                nk = len(w_tiles)
                for kt in range(nk):
                    nc.tensor.matmul(pp[:], w_tiles[kt][:, mt * 128:(mt + 1) * 128],
                                     rhsT[kt][:, h * 512:(h + 1) * 512],
                                     start=(kt == 0), stop=(kt == nk - 1))
                if bias_fn is None:
                    self.evict(o[:, h * 512:(h + 1) * 512], pp[:])
                else:
                    self.evict_bias(o[:, h * 512:(h + 1) * 512], pp[:], bias_fn(mt))
            outs.append(o)
        return outs

    def load_w(self, dram_ap, n_kt, width, tag, pool):
        tiles = []
        for kt in range(n_kt):
            w = pool.tile([128, width], BF16, tag=tag)
            self.nc.sync.dma_start(w[:], dram_ap[kt * 128:(kt + 1) * 128, :])
            tiles.append(w)
        return tiles

    def load_cols(self, dram_ap, n_cols, dt=F32):
        t_ = self.colp.tile([128, n_cols], dt, tag=f"cols{n_cols}")
        self.nc.sync.dma_start(t_[:], dram_ap.rearrange("(c p) -> p c", p=128))
        return t_

    def transpose_V(self, vT, n_tt, vpool):
        nc = self.nc
        V = vpool.tile([128, n_tt, D], BF16, tag="V")
        for tt in range(n_tt):
            pq = self.pst.tile([128, 4, 128], BF16, tag="pst")
            for ft in range(KT):
                nc.tensor.transpose(pq[:, ft, :], vT[ft][:, tt * 128:(tt + 1) * 128],
                                    self.ident[:])
            self.evict(V[:, tt, :], pq[:])
        return V

    def add_residual(self, yT, n_tt):
        nc = self.nc
        for tt in range(n_tt):
            pq = self.pst.tile([128, 4, 128], BF16, tag="pst")
            for ft in range(KT):
                nc.tensor.transpose(pq[:, ft, :], yT[ft][:, tt * 128:(tt + 1) * 128],
                                    self.ident[:])
            nc.vector.tensor_tensor(out=self.x[:, tt, :], in0=self.x[:, tt, :],
                                    in1=pq[:], op=ALU.add)

    def softmax_rows(self, pp, pool=None):
        # scores are bounded (|s| < ~4) so no max-subtraction is needed
        nc = self.nc
        pool = pool or self.ep
        sums = self.svp.tile([128, 1], F32, tag="sums")
        E = pool.tile([128, 512], BF16, tag="E")
        nc.scalar.activation(E[:], pp[:], AF.Exp, accum_out=sums[:])
        rec = self.svp.tile([128, 1], F32, tag="rec")
        nc.vector.reciprocal(rec[:], sums[:])
        A = pool.tile([128, 512], BF16, tag="E")
        nc.vector.tensor_scalar_mul(A[:], E[:], rec[:, 0:1])
        return A

    # ------------------------------------------------------------------
    def attention(self, qT, kT, vT, wo_tiles, bo_c):
        nc, tc = self.nc, self.tc
        with tc.tile_pool(name="vpool", bufs=1) as vpool, \
             tc.tile_pool(name="atp", bufs=8) as atp, \
             tc.tile_pool(name="epA", bufs=16) as epA:
            V = self.transpose_V(vT, 8, vpool)
            OT = [self.actT.tile([128, T], BF16, tag="actT", name=f"OT{i}")
                  for i in range(KT)]
            for b in range(B):
                ps_o = None
                for h in range(HEADS):
                    ft, sub = h // 2, h % 2
                    po = sub * 64
                    Es = []
                    for qt in range(4):
                        pp = self.ps.tile([128, 512], F32, tag="ps")
                        nc.tensor.matmul(
                            pp[:],
                            qT[ft][po:po + 64,
                                   b * 512 + qt * 128: b * 512 + (qt + 1) * 128],
                            kT[ft][po:po + 64, b * 512:(b + 1) * 512],
                            start=True, stop=True)
                        Es.append(self.softmax_rows(pp, epA))
                    ATt = []
                    for kt in range(4):
                        pq = self.pst.tile([128, 4, 128], BF16, tag="pst")
                        for qt in range(4):
                            nc.tensor.transpose(pq[:, qt, :],
                                                Es[qt][:, kt * 128:(kt + 1) * 128],
                                                self.ident[:])
                        at = atp.tile([128, 512], BF16, tag="AT")
                        self.evict(at[:], pq[:])
                        ATt.append(at)
                    if sub == 0:
                        ps_o = self.ps.tile([128, 512], F32, tag="ps")
                    for jt in range(4):
                        nc.tensor.matmul(ps_o[po:po + 64, :],
                                         V[:, b * 4 + jt, h * 64:(h + 1) * 64],
                                         ATt[jt][:], start=(jt == 0), stop=(jt == 3))
                    if sub == 1:
                        self.evict(OT[ft][:, b * 512:(b + 1) * 512], ps_o[:])
            return self.mm_to_T(wo_tiles, OT, KT, T,
                                bias_fn=lambda mt: bo_c[:, mt:mt + 1])

    # ------------------------------------------------------------------
    def ffn(self, xnT, l, pre):
        nc, tc, di = self.nc, self.tc, self.di
        with tc.tile_pool(name=f"w4k_{pre}{l}", bufs=4) as wp4096, \
             tc.tile_pool(name=f"ag_{pre}{l}", bufs=16) as agp:
            w1 = self.load_w(di[pre + "_w1"].ap()[l], KT, FF, "w1", wp4096)
            b1c = self.load_cols(di[pre + "_b1"].ap()[l], 32)
            ag = []
            for f in range(16):
                o = agp.tile([128, T], BF16, tag="ag")
                for h in range(2):
                    pa = self.ps.tile([128, 512], F32, tag="ps")
                    pg = self.ps.tile([128, 512], F32, tag="ps")
                    for kt in range(KT):
                        nc.tensor.matmul(pa[:], w1[kt][:, f * 128:(f + 1) * 128],
                                         xnT[kt][:, h * 512:(h + 1) * 512],
                                         start=(kt == 0), stop=(kt == KT - 1))
                    for kt in range(KT):
                        nc.tensor.matmul(pg[:], w1[kt][:, (16 + f) * 128:(17 + f) * 128],
                                         xnT[kt][:, h * 512:(h + 1) * 512],
                                         start=(kt == 0), stop=(kt == KT - 1))
                    a_sb = self.ep.tile([128, 512], BF16, tag="E")
                    self.evict_bias(a_sb[:], pa[:], b1c[:, f:f + 1])
                    g_sb = self.ep.tile([128, 512], BF16, tag="E")
                    nc.scalar.activation(g_sb[:], pg[:], AF.Gelu,
                                         bias=b1c[:, 16 + f:17 + f])
                    nc.vector.tensor_mul(o[:, h * 512:(h + 1) * 512], a_sb[:], g_sb[:])
                ag.append(o)
            w2 = self.load_w(di[pre + "_w2"].ap()[l], FH // 128, D, "w512", self.wp512)
            b2c = self.load_cols(di[pre + "_b2"].ap()[l], 4)
            yT = self.mm_to_T(w2, ag, KT, T, bias_fn=lambda mt: b2c[:, mt:mt + 1])
            self.add_residual(yT, 8)

    # ------------------------------------------------------------------
    def layer(self, l):
        nc, di = self.nc, self.di
        lns = self.lns
        c0 = l * 6
        x_tile = lambda tt: self.x[:, tt, :]

        # cross-attention
        xnT = self.ln_to_T(x_tile, 8,
                           lambda ft, q: lns[ft][:, c0 + 0:c0 + 1],
                           lambda ft, q: lns[ft][:, c0 + 1:c0 + 2])
        with self.tc.tile_pool(name=f"ctxp{l}", bufs=8) as ctxp:
            ctx_tiles = []
            for tt in range(8):
                b, i = tt // 4, tt % 4
                col = b * CT + l * 512 + i * 128
                pp = self.ps.tile([128, 512], F32, tag="ps")
                nc.tensor.matmul(pp[:], self.embT_ctx[:, col:col + 128],
                                 self.w_pe_sb[:], start=True, stop=True)
                ct = ctxp.tile([128, 512], F32, tag="f32t")
                nc.vector.tensor_tensor(out=ct[:], in0=pp[:], in1=self.ctx_bias_bc[:],
                                        op=ALU.add)
                ctx_tiles.append(ct)
            if l == 0:
                self.tap("dbg_embc2", self.embT_ctx[:])
                self.tap_tiles("dbg_ctx0", ctx_tiles)
            cnT = self.ln_to_T(lambda tt: ctx_tiles[tt][:], 8,
                               lambda ft, q: lns[ft][:, c0 + 2:c0 + 3],
                               lambda ft, q: lns[ft][:, c0 + 3:c0 + 4])
        wq = self.load_w(di["ca_wq"].ap()[l], KT, D, "w512", self.wp512)
        wkv = self.load_w(di["ca_wkv"].ap()[l], KT, 2 * D, "wkv", self.wp1024)
        wo = self.load_w(di["ca_wo"].ap()[l], KT, D, "w512", self.wp512)
        bo_c = self.load_cols(di["ca_bo"].ap()[l], 4)
        qT = self.mm_to_T(wq, xnT, KT, T)
        kvT = self.mm_to_T(wkv, cnT, 2 * KT, T)
        if l == 0:
            self.tap_tiles("dbg_xnT", xnT)
            self.tap_tiles("dbg_cnT", cnT)
            self.tap_tiles("dbg_qT", qT)
            self.tap_tiles("dbg_kT", kvT[:KT])
            self.tap_tiles("dbg_vT", kvT[KT:])
        yT = self.attention(qT, kvT[:KT], kvT[KT:], wo, bo_c)
        if l == 0:
            self.tap_tiles("dbg_yT", yT)
        self.add_residual(yT, 8)
        if l == 0:
            self.tap("dbg_x1", self.x[:])

        # cross FFN
        fnT = self.ln_to_T(x_tile, 8,
                           lambda ft, q: lns[ft][:, c0 + 4:c0 + 5],
                           lambda ft, q: lns[ft][:, c0 + 5:c0 + 6])
        self.ffn(fnT, l, "cf")
        if l == 0:
            self.tap("dbg_x2", self.x[:])

        # FiLM self-attention (film vec idx base: l*16; q index = batch here)
        film = self.film
        sa_s, sa_b = l * 16 + 0, l * 16 + 4
        snT = self.ln_to_T(x_tile, 8,
                           lambda ft, q: film[:, sa_s + ft, q:q + 1],
                           lambda ft, q: film[:, sa_b + ft, q:q + 1])
        wq = self.load_w(di["sa_wq"].ap()[l], KT, D, "w512", self.wp512)
        wkv = self.load_w(di["sa_wkv"].ap()[l], KT, 2 * D, "wkv", self.wp1024)
        wo = self.load_w(di["sa_wo"].ap()[l], KT, D, "w512", self.wp512)
        bo_c = self.load_cols(di["sa_bo"].ap()[l], 4)
        qT = self.mm_to_T(wq, snT, KT, T)
        kvT = self.mm_to_T(wkv, snT, 2 * KT, T)
        yT = self.attention(qT, kvT[:KT], kvT[KT:], wo, bo_c)
        self.add_residual(yT, 8)
        if l == 0:
            self.tap("dbg_x3", self.x[:])

        # FiLM FFN
        sf_s, sf_b = l * 16 + 8, l * 16 + 12
        snT = self.ln_to_T(x_tile, 8,
                           lambda ft, q: film[:, sf_s + ft, q:q + 1],
                           lambda ft, q: film[:, sf_b + ft, q:q + 1])
        self.ffn(snT, l, "sf")
        if l == 0:
            self.tap("dbg_x4", self.x[:])

    # ------------------------------------------------------------------
    def decode(self):
        nc, tc, di = self.nc, self.tc, self.di
        lns, film = self.lns, self.film
        x_tile = lambda tt: self.x[:, tt, :]

        cnT = self.ln_to_T(x_tile, 8,
                           lambda ft, q: lns[ft][:, 26:27],
                           lambda ft, q: lns[ft][:, 27:28])
        dwkv = self.load_w(di["dec_wkv"].ap(), KT, 2 * D, "wkv", self.wp1024)
        kvT = self.mm_to_T(dwkv, cnT, 2 * KT, T)
        kTd, vTd = kvT[:KT], kvT[KT:]
        dwq = self.load_w(di["dec_wq"].ap(), KT, D, "w512", self.wp512)
        dwo = self.load_w(di["dec_wo"].ap(), KT, D, "w512", self.wp512)
        dbo_c = self.load_cols(di["dec_bo"].ap(), 4)
        outw_c = self.load_cols(di["out_w"].ap(), 4, dt=BF16)
        outb_sb = self.consts.tile([1, 1], F32, tag="outb")
        nc.sync.dma_start(outb_sb[:], di["out_b"].ap())

        out_view = self.out_dram.ap().rearrange("b (c t) -> b c t", t=512)

        with tc.tile_pool(name="vpool_d", bufs=1) as vpool, \
             tc.tile_pool(name="atp_d", bufs=8) as atp, \
             tc.tile_pool(name="qep", bufs=5) as qep, \
             tc.tile_pool(name="epD", bufs=12) as epD, \
             tc.tile_pool(name="embq", bufs=1) as embqp:
            Vd = self.transpose_V(vTd, 8, vpool)
            embT_q = embqp.tile([128, B * QT], BF16, tag="embq")
            self.build_embT(embT_q, di["pts_q"], B * QT, QT)

            for qc in range(16):
                b = qc // 8
                qhat = []
                for i in range(4):
                    col = qc * 512 + i * 128
                    pp = self.ps.tile([128, 512], F32, tag="ps")
                    nc.tensor.matmul(pp[:], embT_q[:, col:col + 128], self.w_pe_sb[:],
                                     start=True, stop=True)
                    qe = qep.tile([128, 512], F32, tag="qe")
                    nc.vector.tensor_tensor(out=qe[:], in0=pp[:],
                                            in1=self.qe_bias_bc[:], op=ALU.add)
                    qhat.append(qe)
                qnT = self.ln_to_T(lambda i: qhat[i][:], 4,
                                   lambda ft, q: lns[ft][:, 24:25],
                                   lambda ft, q: lns[ft][:, 25:26])
                qTt = self.mm_to_T(dwq, qnT, KT, 512)
                Es = []
                for i in range(4):
                    pp = self.ps.tile([128, 512], F32, tag="ps")
                    for kt in range(KT):
                        nc.tensor.matmul(pp[:], qTt[kt][:, i * 128:(i + 1) * 128],
                                         kTd[kt][:, b * 512:(b + 1) * 512],
                                         start=(kt == 0), stop=(kt == KT - 1))
                    Es.append(self.softmax_rows(pp, epD))
                ATt = []
                for jt in range(4):
                    pq = self.pst.tile([128, 4, 128], BF16, tag="pst")
                    for i in range(4):
                        nc.tensor.transpose(pq[:, i, :], Es[i][:, jt * 128:(jt + 1) * 128],
                                            self.ident[:])
                    at = atp.tile([128, 512], BF16, tag="AT")
                    self.evict(at[:], pq[:])
                    ATt.append(at)
                OTt = []
                for ft in range(KT):
                    po = self.ps.tile([128, 512], F32, tag="ps")
                    for jt in range(4):
                        nc.tensor.matmul(po[:],
                                         Vd[:, b * 4 + jt, ft * 128:(ft + 1) * 128],
                                         ATt[jt][:], start=(jt == 0), stop=(jt == 3))
                    ot = self.actT.tile([128, 512], BF16, tag="actT")
                    self.evict(ot[:], po[:])
                    OTt.append(ot)
                yTt = self.mm_to_T(dwo, OTt, KT, 512,
                                   bias_fn=lambda mt: dbo_c[:, mt:mt + 1])
                ohat = []
                for i in range(4):
                    pq = self.pst.tile([128, 4, 128], BF16, tag="pst")
                    for ft in range(KT):
                        nc.tensor.transpose(pq[:, ft, :], yTt[ft][:, i * 128:(i + 1) * 128],
                                            self.ident[:])
                    o_sb = qep.tile([128, 512], F32, tag="qe")
                    self.evict(o_sb[:], pq[:])
                    ohat.append(o_sb)
                onT = self.ln_to_T(lambda i: ohat[i][:], 4,
                                   lambda ft, q: film[:, 64 + ft, b:b + 1],
                                   lambda ft, q: film[:, 68 + ft, b:b + 1])
                pf = self.psf.tile([128, 512], F32, tag="ps")
                for kt in range(KT):
                    nc.tensor.matmul(pf[0:1, :], outw_c[:, kt:kt + 1], onT[kt][:],
                                     start=(kt == 0), stop=(kt == KT - 1))
                fo = qep.tile([1, 512], F32, tag="fo")
                nc.scalar.activation(fo[:], pf[0:1, :], AF.Identity,
                                     bias=outb_sb[:, 0:1])
                nc.sync.dma_start(out_view[b, qc % 8, :], fo[:])


# ---------------------------------------------------------------------------
# host side
# ---------------------------------------------------------------------------

_NC_CACHE = None


def _get_nc():
    global _NC_CACHE
    if _NC_CACHE is None:
        _NC_CACHE = Ker().nc
    return _NC_CACHE


def _prep_params(params):
    P = {k: np.asarray(v, dtype=np.float32) for k, v in params.items() if k != "layers"}
    L = {k: np.asarray(v, dtype=np.float32) for k, v in params["layers"].items()}
    d = {}
    d["freqs"] = P["freqs"]
    e = np.power(2.0, np.arange(8)).astype(np.float32) * np.pi
    basis = np.zeros((3, 24), np.float32)
    basis[0, :8] = e
    basis[1, 8:16] = e
    basis[2, 16:] = e
    d["basis"] = basis / TWO_PI
    pe_w = P["pe_w"]
    w128 = np.zeros((128, D), np.float32)
    w128[0:24] = pe_w[0:24]
    w128[32:56] = pe_w[24:48]
    w128[64:67] = pe_w[48:51]
    w128[96:97] = P["ve_w"]
    d["w_pe"] = w128.astype(bf16)
    d["ctx_bias"] = P["pe_b"] + P["ve_b"]
    d["qe_bias"] = P["pe_b"]
    d["lat"] = P["latent"]
    scale = (D // HEADS) ** -0.5
    for pre in ("ca", "sa"):
        d[pre + "_wq"] = (L[pre + "_wq"] * scale).astype(bf16)
        d[pre + "_wkv"] = L[pre + "_wkv"].astype(bf16)
        d[pre + "_wo"] = L[pre + "_wo"].astype(bf16)
        d[pre + "_bo"] = L[pre + "_bo"]
    for pre in ("cf", "sf"):
        d[pre + "_w1"] = L[pre + "_w1"].astype(bf16)
        d[pre + "_b1"] = L[pre + "_b1"]
        d[pre + "_w2"] = L[pre + "_w2"].astype(bf16)
        d[pre + "_b2"] = L[pre + "_b2"]
    mats = []
    for l in range(DEPTH):
        mats.append(L["sa_g"][l] * L["sa_ln_s"][l][None, :])
        mats.append(L["sa_g"][l] * L["sa_ln_b"][l][None, :] + L["sa_be"][l])
        mats.append(L["sf_g"][l] * L["sf_ln_s"][l][None, :])
        mats.append(L["sf_g"][l] * L["sf_ln_b"][l][None, :] + L["sf_be"][l])
    mats.append(P["out_g"] * P["out_ln_s"][None, :])
    mats.append(P["out_g"] * P["out_ln_b"][None, :] + P["out_be"])
    d["w_film"] = np.concatenate(mats, axis=1).astype(bf16)
    cols = []
    for l in range(DEPTH):
        cols += [L["ca_ln_s"][l], L["ca_ln_b"][l], L["ca_lnc_s"][l], L["ca_lnc_b"][l],
                 L["cf_ln_s"][l], L["cf_ln_b"][l]]
    cols += [P["dec_ln_s"], P["dec_ln_b"], P["dec_lnc_s"], P["dec_lnc_b"]]
    d["ln_static"] = np.stack(cols, axis=1)
    d["dec_wq"] = (P["dec_wq"] * (D ** -0.5)).astype(bf16)
    d["dec_wkv"] = P["dec_wkv"].astype(bf16)
    d["dec_wo"] = P["dec_wo"].astype(bf16)
    d["dec_bo"] = P["dec_bo"]
    d["out_w"] = P["out_w"][:, 0].astype(bf16)
    d["out_b"] = P["out_b"]
    d["zerob"] = np.zeros((B * QT,), bf16)
    return {k: np.ascontiguousarray(v) for k, v in d.items()}


def kernel(context_points, context_values, queries, alpha, params):
    context_points = np.asarray(context_points, np.float32)
    context_values = np.asarray(context_values, np.float32)
    queries = np.asarray(queries, np.float32)
    alpha = np.asarray(alpha, np.float32)

    nc = _get_nc()
    const = _prep_params(params)

    in_maps = []
    for c in range(NCORES):
        s = slice(c * B, (c + 1) * B)
        m = dict(const)
        m["pts_ctx"] = np.ascontiguousarray(context_points[s])
        m["vals_ctx"] = np.ascontiguousarray(context_values[s, :, 0])
        m["pts_q"] = np.ascontiguousarray(queries[s])
        m["alpha"] = np.ascontiguousarray(alpha[s])
        in_maps.append(m)

    res = bass_utils.run_bass_kernel_spmd(nc, in_maps, core_ids=list(range(NCORES)))
    outs = [res.results[c]["out"].reshape(B, QT, 1) for c in range(NCORES)]
    return np.concatenate(outs, axis=0)


if __name__ == "__main__":
    _get_nc()
    print("built ok, instructions:", len(_NC_CACHE.inst_map))
